# revision 1
# baseline (speedup 1.0000x reference)
"""BitNetLinear Trainium2 kernel (8 NeuronCores, SPMD data-parallel).

y = round(clip(x, +-127*s)/s)*s @ (ternary(W))^T + ternary(b)
with s = exp2(floor(log2(max|x|/127 + eps))) a power of two (global over x).

Sharding: batch dim (8) -> one batch of [4096, 1024] per core.
Host prep: each x shard is transposed/tiled to [nt, P, KC, RSUB] (the PE
contracts over partitions, and pre-tiling makes every device load fully
contiguous); weight/bias are ternary-quantized on host (the reference does
this once in __init__); the ternary weight ships as bf16 [in, out].

Device (single pass over x): each 1 MiB chunk of x^T is loaded once; an
absmax reduction and the quantize+matmul pipeline both read the same
resident tile. Because floor(log2(.)) commutes with max, the global
power-of-two scale is the max of per-core scales, and with overwhelming
probability it equals the scale of the first 262144-sample chunk
(P(mismatch) ~ e^-38 for randn data). So the kernel SPECULATES with
chunk-0's scale and starts the matmul pipeline at once; the full local max
and a 512B AllReduce(max) trail behind on the collectives hardware. At the
end each core checks speculated-scale == global-scale (one binade compare)
and, on mismatch, re-runs the exact quantize+matmul with the global scale
under a conditional branch - so the result is exact for ANY input.

x is quantized to integer-valued bf16 (round-half-even via the +-1.5*2^23
trick); the bf16 matmul with fp32 PSUM accumulation is exact integer
arithmetic (|x_int| <= 127, w in {-1,0,1}, |acc| < 2^24); the result is
scaled by s*gamma_w and the ternary bias is added.
"""

import numpy as np
import ml_dtypes
from contextlib import ExitStack

import concourse.bass as bass
import concourse.mybir as mybir
import concourse.tile as tile
from concourse import bacc, bass_isa, bass_utils

F32 = mybir.dt.float32
BF16 = mybir.dt.bfloat16
I32 = mybir.dt.int32

N_CORES = 8
P = 128
IN_F = 1024
OUT_F = 1024
KC = IN_F // P          # 8 contraction chunks
RSUB = 256              # rows per chunk
ROUND_C = 12582912.0    # 1.5 * 2**23: float32 round-half-even trick
EPS = 1e-8


def _emit_scale_chain(nc, consts, gmax, gamma_sb, mask_t, expc_t, tag):
    """From a [P,1] absmax tile, compute s = exp2(floor(log2(m/127+eps)))
    via exponent masking, 1/s via exponent arithmetic, and c = s*gamma."""
    v_t = consts.tile([P, 1], F32, tag=f"v_{tag}")
    nc.vector.tensor_scalar(
        out=v_t,
        in0=gmax,
        scalar1=float(np.float32(1.0 / 127.0)),
        scalar2=float(np.float32(EPS)),
        op0=mybir.AluOpType.mult,
        op1=mybir.AluOpType.add,
    )
    s_t = consts.tile([P, 1], F32, tag=f"s_{tag}")
    nc.vector.tensor_tensor(
        out=s_t.bitcast(I32),
        in0=v_t.bitcast(I32),
        in1=mask_t,
        op=mybir.AluOpType.bitwise_and,
    )
    inv_t = consts.tile([P, 1], F32, tag=f"inv_{tag}")
    nc.vector.tensor_tensor(
        out=inv_t.bitcast(I32),
        in0=expc_t,
        in1=s_t.bitcast(I32),
        op=mybir.AluOpType.subtract,
    )
    c_t = consts.tile([P, 1], F32, tag=f"c_{tag}")
    nc.vector.tensor_mul(out=c_t, in0=s_t, in1=gamma_sb)
    return s_t, inv_t, c_t


def _emit_phase2(nc, pools, rows, xt, y_rows, w_sb, bias_sb,
                 scale, chunk_hook=None):
    """Quantize x with 1/s, matmul against the ternary weight, scale by c,
    add bias, store y. `scale` is a dict read lazily (its "inv"/"c" tiles
    may be filled by chunk_hook at t==0). The bias-add + y store of each
    output tile are DEFERRED by two chunks so these matmul-coupled ops
    never sit ahead of the (DMA-paced) quantize producers in the DVE FIFO.
    Returns the last emitted instruction per engine.

    Quantize chain (exact): clip-before-round equals round-then-clip since
    the bounds are integers, so
      t1 = max(x*inv, -127); t2 = min(t1, 127) + C; xi = bf16(t2 - C)
    yields round-half-even(clip(x/s)) exactly (+-1.5*2^23 trick; mult by a
    power of two is exact; integer results |.|<=127 are bf16-exact)."""
    xq_pool, t1_pool, t2_pool, xi_pool, yo_pool, ps_pool = pools
    nhalf = OUT_F // 512
    last = {}
    pending = []

    def flush_pending():
        yo, row = pending.pop(0)
        last["DVE"] = nc.vector.tensor_add(out=yo, in0=yo, in1=bias_sb)
        # y stores go out via SWDGE (gpsimd): on the sync ring their
        # data-ready waits would block the sequencer and starve the x loads
        last["POOL"] = nc.gpsimd.dma_start(out=y_rows[row], in_=yo)

    for t in range(rows // RSUB):
        xc = xq_pool.tile([P, KC, RSUB], F32, tag="xc")
        last["SP"] = nc.sync.dma_start(out=xc, in_=xt[t])
        if chunk_hook is not None:
            chunk_hook(t, xc)
        t1 = t1_pool.tile([P, KC, RSUB], F32, tag="t1")
        nc.vector.tensor_scalar(
            out=t1,
            in0=xc,
            scalar1=scale["inv"],
            scalar2=-127.0,
            op0=mybir.AluOpType.mult,
            op1=mybir.AluOpType.max,
        )
        t2 = t2_pool.tile([P, KC, RSUB], F32, tag="t2")
        nc.vector.tensor_scalar(
            out=t2,
            in0=t1,
            scalar1=127.0,
            scalar2=ROUND_C,
            op0=mybir.AluOpType.min,
            op1=mybir.AluOpType.add,
        )
        xi = xi_pool.tile([P, KC, RSUB], BF16, tag="xi")
        nc.vector.tensor_scalar(
            out=xi,
            in0=t2,
            scalar1=ROUND_C,
            scalar2=None,
            op0=mybir.AluOpType.subtract,
        )
        for h in range(RSUB // P):
            ps = ps_pool.tile([P, OUT_F], F32, tag="ps")
            for k in range(KC):
                for n in range(nhalf):
                    last["PE"] = nc.tensor.matmul(
                        ps[:, n * 512 : (n + 1) * 512],
                        lhsT=xi[:, k, h * P : (h + 1) * P],
                        rhs=w_sb[:, k, n * 512 : (n + 1) * 512],
                        start=(k == 0),
                        stop=(k == KC - 1),
                    )
            yo = yo_pool.tile([P, OUT_F], F32, tag="yo")
            last["ACT"] = nc.scalar.activation(
                out=yo,
                in_=ps,
                func=mybir.ActivationFunctionType.Copy,
                bias=0.0,
                scale=scale["c"],
            )
            pending.append((yo, t * (RSUB // P) + h))
            if len(pending) > 10:
                flush_pending()
    while pending:
        flush_pending()
    return last


def build_program(rows: int = 4096, num_cores: int = N_CORES) -> bacc.Bacc:
    assert rows % RSUB == 0
    nc = bacc.Bacc(
        "TRN2",
        target_bir_lowering=False,
        debug=False,
        enable_asserts=False,
        num_devices=num_cores,
    )
    nt = rows // RSUB
    # x shard pre-tiled on host: xt[t, p, c, r] = x[t*RSUB + r, c*P + p],
    # so every chunk load is one fully-contiguous 1 MiB read.
    xt = nc.dram_tensor("xt", (nt, P, KC, RSUB), F32, kind="ExternalInput").ap()
    wq = nc.dram_tensor("wq", (IN_F, OUT_F), BF16, kind="ExternalInput").ap()
    bq = nc.dram_tensor("bq", (1, OUT_F), F32, kind="ExternalInput").ap()
    gq = nc.dram_tensor("gq", (1, 1), F32, kind="ExternalInput").ap()
    y = nc.dram_tensor("y", (rows, OUT_F), F32, kind="ExternalOutput").ap()
    # Collectives cannot target I/O tensors; bounce through internal DRAM.
    cc_in = nc.dram_tensor("cc_in", (P, 1), F32).ap()
    cc_out = nc.dram_tensor("cc_out", (P, 1), F32).ap()
    # dummy collective to pre-warm the ncfw/collectives firmware
    ccw_in = nc.dram_tensor("ccw_in", (P, 1), F32).ap()
    ccw_out = nc.dram_tensor("ccw_out", (P, 1), F32).ap()

    with tile.TileContext(nc, num_cores=num_cores) as tc, ExitStack() as ctx:
        consts = ctx.enter_context(tc.tile_pool(name="consts", bufs=1))

        mask_t = consts.tile([P, 1], I32)
        nc.vector.memset(mask_t, -8388608)  # 0xFF800000: sign+exponent mask
        expc_t = consts.tile([P, 1], I32)
        nc.vector.memset(expc_t, 0x7F000000)  # bits of (254<<23)

        # constants first on the sync ring (the first matmul needs w_sb)
        gamma_sb = consts.tile([P, 1], F32)
        nc.sync.dma_start(out=gamma_sb, in_=gq.to_broadcast((P, 1)))
        w_sb = consts.tile([P, KC, OUT_F], BF16)
        nc.sync.dma_start(out=w_sb, in_=wq.rearrange("(c p) o -> p c o", p=P))
        bias_sb = consts.tile([P, OUT_F], F32)
        nc.sync.dma_start(out=bias_sb, in_=bq.to_broadcast((P, OUT_F)))

        y_rows = y.rearrange("(t p) o -> t p o", p=P)
        partials = consts.tile([P, nt], F32)
        warm_rhs = consts.tile([P, 512], BF16)
        nc.vector.memset(warm_rhs, 0.0)

        scale_spec = {}
        gmax_g = consts.tile([P, 1], F32)
        twos_s = consts.tile([P, 1], F32)
        rg = [list(range(num_cores))]

        with (
            tc.tile_pool(name="xq", bufs=6) as xq_pool,
            tc.tile_pool(name="t1", bufs=2) as t1_pool,
            tc.tile_pool(name="t2", bufs=2) as t2_pool,
            tc.tile_pool(name="xi", bufs=6) as xi_pool,
            tc.tile_pool(name="yo", bufs=13) as yo_pool,
            tc.tile_pool(name="ps", bufs=4, space="PSUM") as ps_pool,
        ):
            pools = (xq_pool, t1_pool, t2_pool, xi_pool, yo_pool, ps_pool)

            def chunk_hook(t, xc):
                # fused absmax over the chunk just loaded (same read of x),
                # at the head of this chunk's DVE segment (bias deferral
                # keeps these segments close to DMA pace)
                nc.vector.tensor_reduce(
                    out=partials[:, t : t + 1],
                    in_=xc,
                    axis=mybir.AxisListType.XY,
                    op=mybir.AluOpType.max,
                    apply_absolute_value=True,
                )
                if t == 0:
                    # speculative scale from chunk 0 alone
                    lmax_s = consts.tile([P, 1], F32)
                    nc.vector.tensor_copy(out=lmax_s, in_=partials[:, 0:1])
                    gmax_s = consts.tile([P, 1], F32)
                    nc.gpsimd.partition_all_reduce(
                        gmax_s, lmax_s, channels=P,
                        reduce_op=bass_isa.ReduceOp.max,
                    )
                    s_s, inv_s, c_s = _emit_scale_chain(
                        nc, consts, gmax_s, gamma_sb, mask_t, expc_t, "l")
                    scale_spec["inv"] = inv_s
                    scale_spec["c"] = c_s
                    nc.vector.tensor_scalar(
                        out=twos_s, in0=s_s, scalar1=2.0, scalar2=None,
                        op0=mybir.AluOpType.mult,
                    )
                    # pre-warm the collectives firmware with a dummy 512B
                    # AllReduce so the real one later starts without the
                    # ncfw cold-wake latency
                    nc.gpsimd.dma_start(out=ccw_in, in_=gmax_s)
                    nc.gpsimd.collective_compute(
                        "AllReduce", mybir.AluOpType.max, replica_groups=rg,
                        ins=[ccw_in.opt()], outs=[ccw_out.opt()],
                    )
                    # PE warmup: junk matmuls flip HAM to full clock while
                    # the first quantize runs
                    nc.vector.tensor_copy(out=warm_rhs[:, 0:1], in_=gmax_s)
                    warm_ps = ps_pool.tile([P, OUT_F], F32, tag="ps")
                    for _ in range(10):
                        nc.tensor.matmul(
                            warm_ps[:, 0:512], lhsT=warm_rhs[:, 0:P],
                            rhs=warm_rhs, start=True, stop=True,
                        )
                if t == nt - 1:
                    # full local max -> 512B AllReduce(max) across the 8
                    # cores; trails under the matmul phase, only the final
                    # speculation check waits on it
                    lmax = consts.tile([P, 1], F32)
                    nc.vector.tensor_reduce(
                        out=lmax, in_=partials, axis=mybir.AxisListType.X,
                        op=mybir.AluOpType.max,
                    )
                    gmax_l = consts.tile([P, 1], F32)
                    nc.gpsimd.partition_all_reduce(
                        gmax_l, lmax, channels=P,
                        reduce_op=bass_isa.ReduceOp.max,
                    )
                    nc.gpsimd.dma_start(out=cc_in, in_=gmax_l)
                    nc.gpsimd.collective_compute(
                        "AllReduce", mybir.AluOpType.max, replica_groups=rg,
                        ins=[cc_in.opt()], outs=[cc_out.opt()],
                    )
                    nc.gpsimd.dma_start(out=gmax_g, in_=cc_out)

            last = _emit_phase2(
                nc, pools, rows, xt, y_rows, w_sb, bias_sb,
                scale_spec, chunk_hook=chunk_hook,
            )

            # --- verify the speculation: the speculative scale is the global
            # one iff v_g = gmax_g/127+eps stays in the same binade, i.e.
            # v_g - 2*s_spec < 0. On DVE at the end of its stream (gpsimd
            # op-type switches cost ~5us Q7 library reloads each). ---
            v_g2 = consts.tile([P, 1], F32)
            nc.vector.tensor_scalar(
                out=v_g2,
                in0=gmax_g,
                scalar1=float(np.float32(1.0 / 127.0)),
                scalar2=float(np.float32(EPS)),
                op0=mybir.AluOpType.mult,
                op1=mybir.AluOpType.add,
            )
            chk = consts.tile([P, 1], F32)
            last["DVE"] = nc.vector.tensor_tensor(
                out=chk, in0=v_g2, in1=twos_s,
                op=mybir.AluOpType.subtract,
            )
            regs = nc.alloc_registers(
                "spec_chk",
                bass.OrderedSet([
                    mybir.EngineType.SP,
                    mybir.EngineType.DVE,
                    mybir.EngineType.Activation,
                    mybir.EngineType.PE,
                    mybir.EngineType.Pool,
                ]),
            )
            # Pin each engine's reg_load after its last speculative-phase
            # instruction: the load waits on the AllReduce, and the Tile
            # scheduler would otherwise be free to place it mid-stream,
            # stalling that engine's FIFO on the collective.
            eng_key = {
                mybir.EngineType.PE: "PE",
                mybir.EngineType.DVE: "DVE",
                mybir.EngineType.Activation: "ACT",
                mybir.EngineType.SP: "SP",
                mybir.EngineType.Pool: "POOL",
            }
            for reg in regs:
                ld = nc.reg_load(reg, chk.bitcast(I32)[0:1, 0:1])
                prev = last.get(eng_key[reg.engine])
                if prev is not None:
                    tile.add_dep_helper(
                        ld.ins, prev.ins, sync=False,
                        reason="speculation check after spec phase",
                    )
            # f32 bits as int32: negative iff chk < 0 (fast path)
            with tc.If(nc.snap(regs) >= 0):
                # mismatch: redo everything with the global scale
                _, inv_g, c_g = _emit_scale_chain(
                    nc, consts, gmax_g, gamma_sb, mask_t, expc_t, "g")
                _emit_phase2(nc, pools, rows, xt, y_rows, w_sb, bias_sb,
                             {"inv": inv_g, "c": c_g})

    nc.compile()
    return nc


def quantize_params(weight: np.ndarray, bias: np.ndarray):
    """Ternary-quantize weight/bias exactly as the reference (f64 math whose
    f32 rounding matches jax-f32; verified margins are orders of magnitude
    above f32 accumulation differences)."""
    w64 = weight.astype(np.float64)
    g_w = np.float32(np.abs(w64).mean())
    wi = np.clip(np.round(w64 / (np.float64(g_w) + EPS)), -1.0, 1.0)
    b64 = bias.astype(np.float64)
    g_b = np.float32(np.abs(b64).mean())
    bi = np.clip(np.round(b64 / (np.float64(g_b) + EPS)), -1.0, 1.0)
    bq = (bi * np.float64(g_b)).astype(np.float32)  # exact: {-g_b, 0, g_b}
    return wi, g_w, bq


_PROGRAM_CACHE: dict[int, bacc.Bacc] = {}


def _get_program(rows: int) -> bacc.Bacc:
    if rows not in _PROGRAM_CACHE:
        _PROGRAM_CACHE[rows] = build_program(rows)
    return _PROGRAM_CACHE[rows]


def tile_x_shard(x2d: np.ndarray) -> np.ndarray:
    """[rows, IN_F] -> [nt, P, KC, RSUB] with xt[t,p,c,r] = x[t*RSUB+r, c*P+p]."""
    rows = x2d.shape[0]
    return np.ascontiguousarray(
        x2d.reshape(rows // RSUB, RSUB, KC, P).transpose(0, 3, 2, 1)
    )


def prepare_in_maps(x: np.ndarray, weight: np.ndarray, bias: np.ndarray):
    x = np.asarray(x, dtype=np.float32)
    weight = np.asarray(weight, dtype=np.float32)
    bias = np.asarray(bias, dtype=np.float32)
    batch, rows, in_f = x.shape
    assert batch == N_CORES and in_f == IN_F and weight.shape == (OUT_F, IN_F)

    wi, g_w, bq = quantize_params(weight, bias)
    wq_t = np.ascontiguousarray(wi.T).astype(ml_dtypes.bfloat16)  # [in, out]
    bq_row = np.ascontiguousarray(bq.reshape(1, OUT_F))
    gq = np.array([[g_w]], dtype=np.float32)

    in_maps = []
    for c in range(N_CORES):
        in_maps.append(
            {
                "xt": tile_x_shard(x[c]),
                "wq": wq_t,
                "bq": bq_row,
                "gq": gq,
            }
        )
    return in_maps, rows


def kernel(x: np.ndarray, weight: np.ndarray, bias: np.ndarray) -> np.ndarray:
    in_maps, rows = prepare_in_maps(x, weight, bias)
    nc = _get_program(rows)
    res = bass_utils.run_bass_kernel_spmd(nc, in_maps, core_ids=list(range(N_CORES)))
    return np.stack([res.results[c]["y"] for c in range(N_CORES)], axis=0)



# revision 2
# speedup vs baseline: 1.0100x; 1.0100x over previous
"""BitNetLinear Trainium2 kernel (8 NeuronCores, SPMD data-parallel).

y = round(clip(x, +-127*s)/s)*s @ (ternary(W))^T + ternary(b)
with s = exp2(floor(log2(max|x|/127 + eps))) a power of two (global over x).

Sharding: batch dim (8) -> one batch of [4096, 1024] per core.

v2 design (vs the 208us v1):
 * TRANSPOSED GEMM: compute y^T with out_features on PSUM partitions
   (lhsT = W^T block stationary, quantized-x rows streaming). The ternary
   bias becomes per-partition and fuses into the ACT-engine PSUM->SBUF
   copy (activation Identity: out = ps*c + b) - no DVE bias pass, stores
   depend only on PE+ACT. Host transposes y back (layout only).
 * HARDCODED SPECULATIVE SCALE: for this input regime (randn) the global
   power-of-two scale is 2^-5 with overwhelming probability
   (P(other binade) < 1e-7 for any randn(0,1) of this size). The kernel
   runs the whole pipeline with s_spec = 2^-5 baked in, so the first
   matmul starts as soon as chunk 0 lands - no absmax on the critical
   path. Exactness is unconditional: per-chunk absmaxes accumulate off
   the critical path, a 512B AllReduce(max) produces the true global
   max, and a one-instruction binade check (sign(v-s) XOR sign(v-2s))
   branches to an exact full redo with the device-computed scale if the
   speculation missed (ANY input remains bit-correct, just slower).
 * EAGER x RESIDENCY: all 16 MiB of the x shard is loaded up front into
   SBUF (it fits), so loads never pace compute, the AllReduce finishes
   long before the last matmul, and the redo path (if taken) reads x
   from SBUF without reloading.

x is quantized to integer-valued bf16 (round-half-even via the +-1.5*2^23
trick); the bf16 matmul with fp32 PSUM accumulation is exact integer
arithmetic (|x_int| <= 127, w in {-1,0,1}, |acc| < 2^24); the result is
scaled by s*gamma_w and the ternary bias added, all in the ACT copy.
"""

import numpy as np
import ml_dtypes
from contextlib import ExitStack

import concourse.bass as bass
import concourse.mybir as mybir
import concourse.tile as tile
from concourse import bacc, bass_isa, bass_utils

F32 = mybir.dt.float32
BF16 = mybir.dt.bfloat16
I32 = mybir.dt.int32

N_CORES = 8
P = 128
IN_F = 1024
OUT_F = 1024
KC = IN_F // P          # 8 contraction chunks of 128
JC = OUT_F // P         # 8 output blocks of 128
RSUB = 512              # rows per chunk / row-group
ROUND_C = 12582912.0    # 1.5 * 2**23: float32 round-half-even trick
EPS = 1e-8
S_SPEC = 2.0 ** -5      # speculative global scale (binade of max|x|/127+eps)
INV_SPEC = 2.0 ** 5


def _emit_scale_chain(nc, consts, gmax, gamma_sb, mask_t, expc_t, tag):
    """From a [P,1] absmax tile, compute s = exp2(floor(log2(m/127+eps)))
    via exponent masking, 1/s via exponent arithmetic, and c = s*gamma."""
    v_t = consts.tile([P, 1], F32, tag=f"v_{tag}")
    nc.vector.tensor_scalar(
        out=v_t,
        in0=gmax,
        scalar1=float(np.float32(1.0 / 127.0)),
        scalar2=float(np.float32(EPS)),
        op0=mybir.AluOpType.mult,
        op1=mybir.AluOpType.add,
    )
    s_t = consts.tile([P, 1], F32, tag=f"s_{tag}")
    nc.vector.tensor_tensor(
        out=s_t.bitcast(I32),
        in0=v_t.bitcast(I32),
        in1=mask_t,
        op=mybir.AluOpType.bitwise_and,
    )
    inv_t = consts.tile([P, 1], F32, tag=f"inv_{tag}")
    nc.vector.tensor_tensor(
        out=inv_t.bitcast(I32),
        in0=expc_t,
        in1=s_t.bitcast(I32),
        op=mybir.AluOpType.subtract,
    )
    c_t = consts.tile([P, 1], F32, tag=f"c_{tag}")
    nc.vector.tensor_mul(out=c_t, in0=s_t, in1=gamma_sb)
    return s_t, inv_t, c_t


def _emit_phase(nc, pools, nt, xc_tiles, yT, w_sb, bias_sb, inv, c_scale,
                rg_hook=None):
    """Quantize x with 1/s (DVE mult+max, min+addC; ACT subC->bf16), then
    per row-group run the transposed matmul (W^T blocks stationary, xi rows
    streaming, PSUM partition dim = out_features), fuse scale+bias into the
    ACT PSUM->SBUF copy, store y^T tiles. `inv` is a float or [P,1] tile;
    `c_scale` a [P,1] tile. Returns last emitted instruction per engine.

    Quantize chain (exact): clip-before-round equals round-then-clip since
    the bounds are integers, so
      t1 = max(x*inv, -127); t1 = min(t1, 127) + C; xi = bf16(t1 - C)
    yields round-half-even(clip(x/s)) exactly (+-1.5*2^23 trick; mult by a
    power of two is exact; integer results |.|<=127 are bf16-exact)."""
    t1_pool, xi_pool, yo_pool, ps_pool = pools
    last = {}
    for rg in range(nt):
        xc = xc_tiles[rg]
        xi_slices = []
        for h in range(2):
            t1 = t1_pool.tile([P, KC // 2, RSUB], F32, tag="t1")
            nc.vector.tensor_scalar(
                out=t1,
                in0=xc[:, h * (KC // 2): (h + 1) * (KC // 2), :],
                scalar1=inv,
                scalar2=-127.0,
                op0=mybir.AluOpType.mult,
                op1=mybir.AluOpType.max,
            )
            last["DVE"] = nc.vector.tensor_scalar(
                out=t1,
                in0=t1,
                scalar1=127.0,
                scalar2=ROUND_C,
                op0=mybir.AluOpType.min,
                op1=mybir.AluOpType.add,
            )
            for kk in range(KC // 2):
                xi = xi_pool.tile([P, RSUB], BF16, tag="xi")
                last["ACT"] = nc.scalar.activation(
                    out=xi,
                    in_=t1[:, kk, :],
                    func=mybir.ActivationFunctionType.Copy,
                    bias=-ROUND_C,
                    scale=1.0,
                )
                xi_slices.append(xi)
        if rg_hook is not None:
            rg_hook(rg, xc)
        for j in range(JC):
            ps = ps_pool.tile([P, RSUB], F32, tag="ps")
            for k in range(KC):
                last["PE"] = nc.tensor.matmul(
                    ps,
                    lhsT=w_sb[:, k, j * P: (j + 1) * P],
                    rhs=xi_slices[k],
                    start=(k == 0),
                    stop=(k == KC - 1),
                )
            yo = yo_pool.tile([P, RSUB], F32, tag="yo")
            last["ACT"] = nc.scalar.activation(
                out=yo,
                in_=ps,
                func=mybir.ActivationFunctionType.Identity,
                bias=bias_sb[:, j: j + 1],
                scale=c_scale,
            )
            last["POOL"] = nc.gpsimd.dma_start(
                out=yT[j * P: (j + 1) * P, rg * RSUB: (rg + 1) * RSUB],
                in_=yo,
            )
    return last


def build_program(rows: int = 4096, num_cores: int = N_CORES) -> bacc.Bacc:
    assert rows % RSUB == 0
    nc = bacc.Bacc(
        "TRN2",
        target_bir_lowering=False,
        debug=False,
        enable_asserts=False,
        num_devices=num_cores,
    )
    nt = rows // RSUB
    # x shard pre-tiled on host: xt[t, p, c, r] = x[t*RSUB + r, c*P + p],
    # so every chunk load is fully contiguous.
    xt = nc.dram_tensor("xt", (nt, P, KC, RSUB), F32, kind="ExternalInput").ap()
    wq = nc.dram_tensor("wq", (IN_F, OUT_F), BF16, kind="ExternalInput").ap()
    # bias pre-transposed on host to [P, JC]: bqt[p, j] = bq[j*128 + p]
    bqt = nc.dram_tensor("bqt", (P, JC), F32, kind="ExternalInput").ap()
    gq = nc.dram_tensor("gq", (1, 1), F32, kind="ExternalInput").ap()
    # transposed output y^T [out_features, rows]; host transposes back
    yT = nc.dram_tensor("yT", (OUT_F, rows), F32, kind="ExternalOutput").ap()
    # Collectives cannot target I/O tensors; bounce through internal DRAM.
    cc_in = nc.dram_tensor("cc_in", (P, 1), F32).ap()
    cc_out = nc.dram_tensor("cc_out", (P, 1), F32).ap()
    # dummy collective to pre-warm the ncfw/collectives firmware
    ccw_in = nc.dram_tensor("ccw_in", (P, 1), F32).ap()
    ccw_out = nc.dram_tensor("ccw_out", (P, 1), F32).ap()

    with tile.TileContext(nc, num_cores=num_cores) as tc, ExitStack() as ctx:
        consts = ctx.enter_context(tc.tile_pool(name="consts", bufs=1))

        mask_t = consts.tile([P, 1], I32)
        nc.vector.memset(mask_t, -8388608)  # 0xFF800000: sign+exponent mask
        expc_t = consts.tile([P, 1], I32)
        nc.vector.memset(expc_t, 0x7F000000)  # bits of (254<<23)

        # constant loads on the ACT (scalar) ring: the sync ring is fully
        # dedicated to the eager x loads so chunk 0 lands as early as
        # possible, while the weight rides a parallel queue.
        gamma_sb = consts.tile([P, 1], F32)
        nc.scalar.dma_start(out=gamma_sb, in_=gq.to_broadcast((P, 1)))
        w_sb = consts.tile([P, KC, OUT_F], BF16)
        nc.scalar.dma_start(out=w_sb, in_=wq.rearrange("(c p) o -> p c o", p=P))
        bias_sb = consts.tile([P, JC], F32)
        nc.scalar.dma_start(out=bias_sb, in_=bqt)

        # c = s_spec * gamma for the speculative fast path
        c_spec = consts.tile([P, 1], F32)
        nc.vector.tensor_scalar(
            out=c_spec, in0=gamma_sb, scalar1=S_SPEC, scalar2=None,
            op0=mybir.AluOpType.mult,
        )

        partials = consts.tile([P, nt], F32)
        gmax_g = consts.tile([P, 1], F32)
        warm_rhs = consts.tile([P, RSUB], BF16)
        nc.vector.memset(warm_rhs, 0.0)
        warm_f = consts.tile([P, 1], F32)
        nc.vector.memset(warm_f, 1.0)
        warm_o = consts.tile([P, 1], F32)
        rg_cc = [list(range(num_cores))]

        with (
            tc.tile_pool(name="xc", bufs=nt) as xc_pool,
            tc.tile_pool(name="t1", bufs=2) as t1_pool,
            tc.tile_pool(name="xi", bufs=2 * KC) as xi_pool,
            tc.tile_pool(name="yo", bufs=6) as yo_pool,
            tc.tile_pool(name="ps", bufs=8, space="PSUM") as ps_pool,
        ):
            pools = (t1_pool, xi_pool, yo_pool, ps_pool)

            # eager x loads: all chunks issued up front on the sync ring,
            # two half-chunk DMAs per chunk (finer landing granularity).
            xc_tiles = []
            last_sp = None
            for t in range(nt):
                xc = xc_pool.tile([P, KC, RSUB], F32, tag="xc")
                for h in range(2):
                    last_sp = nc.sync.dma_start(
                        out=xc[:, h * (KC // 2): (h + 1) * (KC // 2), :],
                        in_=xt[t, :, h * (KC // 2): (h + 1) * (KC // 2), :],
                    )
                xc_tiles.append(xc)

            # --- warmups, all dependency-free ---
            # PE: junk matmuls flip HAM to full clock before the first
            # real matmul
            warm_ps = ps_pool.tile([P, RSUB], F32, tag="ps")
            for _ in range(10):
                nc.tensor.matmul(
                    warm_ps, lhsT=warm_rhs[:, 0:P], rhs=warm_rhs,
                    start=True, stop=True,
                )
            # gpsimd: pre-load the Q7 partition-reduce library
            nc.gpsimd.partition_all_reduce(
                warm_o, warm_f, channels=P, reduce_op=bass_isa.ReduceOp.max,
            )
            # collectives firmware: dummy 512B AllReduce so the real one
            # later starts without the ncfw cold-wake latency
            nc.gpsimd.dma_start(out=ccw_in, in_=warm_f)
            nc.gpsimd.collective_compute(
                "AllReduce", mybir.AluOpType.max, replica_groups=rg_cc,
                ins=[ccw_in.opt()], outs=[ccw_out.opt()],
            )

            def rg_hook(rg, xc):
                # per-chunk absmax, off the critical path (reads the
                # resident x chunk); placed in the DVE stream inside rg's
                # segment so it never heads-of-line-blocks on a load
                nc.vector.tensor_reduce(
                    out=partials[:, rg: rg + 1],
                    in_=xc,
                    axis=mybir.AxisListType.XY,
                    op=mybir.AluOpType.max,
                    apply_absolute_value=True,
                )
                if rg == nt - 1:
                    # full local max -> 512B AllReduce(max) across the 8
                    # cores; completes well before the last matmul
                    lmax = consts.tile([P, 1], F32)
                    nc.vector.tensor_reduce(
                        out=lmax, in_=partials, axis=mybir.AxisListType.X,
                        op=mybir.AluOpType.max,
                    )
                    gmax_l = consts.tile([P, 1], F32)
                    nc.gpsimd.partition_all_reduce(
                        gmax_l, lmax, channels=P,
                        reduce_op=bass_isa.ReduceOp.max,
                    )
                    nc.gpsimd.dma_start(out=cc_in, in_=gmax_l)
                    nc.gpsimd.collective_compute(
                        "AllReduce", mybir.AluOpType.max, replica_groups=rg_cc,
                        ins=[cc_in.opt()], outs=[cc_out.opt()],
                    )
                    nc.gpsimd.dma_start(out=gmax_g, in_=cc_out)

            last = _emit_phase(
                nc, pools, nt, xc_tiles, yT, w_sb, bias_sb,
                INV_SPEC, c_spec, rg_hook=rg_hook,
            )
            last["SP"] = last_sp

            # --- speculation check: s_spec is the true scale iff
            # v = gmax/127+eps lies in [s_spec, 2*s_spec), i.e.
            # sign(v - s) != sign(v - 2s). XOR of the float bits makes the
            # pass condition a single sign test: chk < 0 <=> in-binade. ---
            v_g = consts.tile([P, 1], F32)
            nc.vector.tensor_scalar(
                out=v_g,
                in0=gmax_g,
                scalar1=float(np.float32(1.0 / 127.0)),
                scalar2=float(np.float32(EPS)),
                op0=mybir.AluOpType.mult,
                op1=mybir.AluOpType.add,
            )
            a_t = consts.tile([P, 1], F32)
            nc.vector.tensor_scalar(
                out=a_t, in0=v_g, scalar1=-S_SPEC, scalar2=None,
                op0=mybir.AluOpType.add,
            )
            b_t = consts.tile([P, 1], F32)
            nc.vector.tensor_scalar(
                out=b_t, in0=v_g, scalar1=-2.0 * S_SPEC, scalar2=None,
                op0=mybir.AluOpType.add,
            )
            chk = consts.tile([P, 1], I32)
            last["DVE"] = nc.vector.tensor_tensor(
                out=chk, in0=a_t.bitcast(I32), in1=b_t.bitcast(I32),
                op=mybir.AluOpType.bitwise_xor,
            )
            regs = nc.alloc_registers(
                "spec_chk",
                bass.OrderedSet([
                    mybir.EngineType.SP,
                    mybir.EngineType.DVE,
                    mybir.EngineType.Activation,
                    mybir.EngineType.PE,
                    mybir.EngineType.Pool,
                ]),
            )
            # Pin each engine's reg_load after its last speculative-phase
            # instruction: the load waits on the AllReduce, and the Tile
            # scheduler would otherwise be free to place it mid-stream,
            # stalling that engine's FIFO on the collective.
            eng_key = {
                mybir.EngineType.PE: "PE",
                mybir.EngineType.DVE: "DVE",
                mybir.EngineType.Activation: "ACT",
                mybir.EngineType.SP: "SP",
                mybir.EngineType.Pool: "POOL",
            }
            for reg in regs:
                ld = nc.reg_load(reg, chk[0:1, 0:1])
                prev = last.get(eng_key[reg.engine])
                if prev is not None:
                    tile.add_dep_helper(
                        ld.ins, prev.ins, sync=False,
                        reason="speculation check after spec phase",
                    )
            # negative iff inside the speculated binade (fast path)
            with tc.If(nc.snap(regs) >= 0):
                # mismatch: redo everything with the exact global scale
                # (x chunks are still resident in SBUF - no reloads)
                _, inv_g, c_g = _emit_scale_chain(
                    nc, consts, gmax_g, gamma_sb, mask_t, expc_t, "g")
                _emit_phase(nc, pools, nt, xc_tiles, yT, w_sb, bias_sb,
                            inv_g, c_g)

    nc.compile()
    return nc


def quantize_params(weight: np.ndarray, bias: np.ndarray):
    """Ternary-quantize weight/bias exactly as the reference (f64 math whose
    f32 rounding matches jax-f32; verified margins are orders of magnitude
    above f32 accumulation differences)."""
    w64 = weight.astype(np.float64)
    g_w = np.float32(np.abs(w64).mean())
    wi = np.clip(np.round(w64 / (np.float64(g_w) + EPS)), -1.0, 1.0)
    b64 = bias.astype(np.float64)
    g_b = np.float32(np.abs(b64).mean())
    bi = np.clip(np.round(b64 / (np.float64(g_b) + EPS)), -1.0, 1.0)
    bq = (bi * np.float64(g_b)).astype(np.float32)  # exact: {-g_b, 0, g_b}
    return wi, g_w, bq


_PROGRAM_CACHE: dict[int, bacc.Bacc] = {}


def _get_program(rows: int) -> bacc.Bacc:
    if rows not in _PROGRAM_CACHE:
        _PROGRAM_CACHE[rows] = build_program(rows)
    return _PROGRAM_CACHE[rows]


def tile_x_shard(x2d: np.ndarray) -> np.ndarray:
    """[rows, IN_F] -> [nt, P, KC, RSUB] with xt[t,p,c,r] = x[t*RSUB+r, c*P+p]."""
    rows = x2d.shape[0]
    return np.ascontiguousarray(
        x2d.reshape(rows // RSUB, RSUB, KC, P).transpose(0, 3, 2, 1)
    )


def prepare_in_maps(x: np.ndarray, weight: np.ndarray, bias: np.ndarray):
    x = np.asarray(x, dtype=np.float32)
    weight = np.asarray(weight, dtype=np.float32)
    bias = np.asarray(bias, dtype=np.float32)
    batch, rows, in_f = x.shape
    assert batch == N_CORES and in_f == IN_F and weight.shape == (OUT_F, IN_F)

    wi, g_w, bq = quantize_params(weight, bias)
    wq_t = np.ascontiguousarray(wi.T).astype(ml_dtypes.bfloat16)  # [in, out]
    bqt = np.ascontiguousarray(bq.reshape(JC, P).T)               # [P, JC]
    gq = np.array([[g_w]], dtype=np.float32)

    in_maps = []
    for c in range(N_CORES):
        in_maps.append(
            {
                "xt": tile_x_shard(x[c]),
                "wq": wq_t,
                "bqt": bqt,
                "gq": gq,
            }
        )
    return in_maps, rows


def kernel(x: np.ndarray, weight: np.ndarray, bias: np.ndarray) -> np.ndarray:
    in_maps, rows = prepare_in_maps(x, weight, bias)
    nc = _get_program(rows)
    res = bass_utils.run_bass_kernel_spmd(nc, in_maps, core_ids=list(range(N_CORES)))
    return np.stack(
        [np.ascontiguousarray(res.results[c]["yT"].T) for c in range(N_CORES)],
        axis=0,
    )


# revision 10
# speedup vs baseline: 1.0363x; 1.0260x over previous
"""BitNetLinear Trainium2 kernel (8 NeuronCores, SPMD data-parallel).

y = round(clip(x, +-127*s)/s)*s @ (ternary(W))^T + ternary(b)
with s = exp2(floor(log2(max|x|/127 + eps))) a power of two (global over x).

Sharding: batch dim (8) -> one batch of [4096, 1024] per core.

v2 design (vs the 208us v1):
 * TRANSPOSED GEMM: compute y^T with out_features on PSUM partitions
   (lhsT = W^T block stationary, quantized-x rows streaming). The ternary
   bias becomes per-partition and fuses into the ACT-engine PSUM->SBUF
   copy (activation Identity: out = ps*c + b) - no DVE bias pass, stores
   depend only on PE+ACT. Host transposes y back (layout only).
 * HARDCODED SPECULATIVE SCALE: for this input regime (randn) the global
   power-of-two scale is 2^-5 with overwhelming probability
   (P(other binade) < 1e-7 for any randn(0,1) of this size). The kernel
   runs the whole pipeline with s_spec = 2^-5 baked in, so the first
   matmul starts as soon as chunk 0 lands - no absmax on the critical
   path. Exactness is unconditional: per-chunk absmaxes accumulate off
   the critical path, a 512B AllReduce(max) produces the true global
   max, and a one-instruction binade check (sign(v-s) XOR sign(v-2s))
   branches to an exact full redo with the device-computed scale if the
   speculation missed (ANY input remains bit-correct, just slower).
 * EAGER x RESIDENCY: all 16 MiB of the x shard is loaded up front into
   SBUF (it fits), so loads never pace compute, the AllReduce finishes
   long before the last matmul, and the redo path (if taken) reads x
   from SBUF without reloading.

x is quantized to integer-valued bf16 (round-half-even via the +-1.5*2^23
trick); the bf16 matmul with fp32 PSUM accumulation is exact integer
arithmetic (|x_int| <= 127, w in {-1,0,1}, |acc| < 2^24); the result is
scaled by s*gamma_w and the ternary bias added, all in the ACT copy.
"""

import numpy as np
import ml_dtypes
from contextlib import ExitStack

import concourse.bass as bass
import concourse.mybir as mybir
import concourse.tile as tile
from concourse import bacc, bass_isa, bass_utils

F32 = mybir.dt.float32
BF16 = mybir.dt.bfloat16
I32 = mybir.dt.int32

N_CORES = 8
P = 128
IN_F = 1024
OUT_F = 1024
KC = IN_F // P          # 8 contraction chunks of 128
JC = OUT_F // P         # 8 output blocks of 128
RSUB = 512              # rows per chunk / row-group
ROUND_C = 12582912.0    # 1.5 * 2**23: float32 round-half-even trick
EPS = 1e-8
S_SPEC = 2.0 ** -5      # speculative global scale (binade of max|x|/127+eps)
INV_SPEC = 2.0 ** 5


def _emit_scale_chain(nc, consts, gmax, gamma_sb, mask_t, expc_t, tag):
    """From a [P,1] absmax tile, compute s = exp2(floor(log2(m/127+eps)))
    via exponent masking, 1/s via exponent arithmetic, and c = s*gamma."""
    v_t = consts.tile([P, 1], F32, tag=f"v_{tag}")
    nc.vector.tensor_scalar(
        out=v_t,
        in0=gmax,
        scalar1=float(np.float32(1.0 / 127.0)),
        scalar2=float(np.float32(EPS)),
        op0=mybir.AluOpType.mult,
        op1=mybir.AluOpType.add,
    )
    s_t = consts.tile([P, 1], F32, tag=f"s_{tag}")
    nc.vector.tensor_tensor(
        out=s_t.bitcast(I32),
        in0=v_t.bitcast(I32),
        in1=mask_t,
        op=mybir.AluOpType.bitwise_and,
    )
    inv_t = consts.tile([P, 1], F32, tag=f"inv_{tag}")
    nc.vector.tensor_tensor(
        out=inv_t.bitcast(I32),
        in0=expc_t,
        in1=s_t.bitcast(I32),
        op=mybir.AluOpType.subtract,
    )
    c_t = consts.tile([P, 1], F32, tag=f"c_{tag}")
    nc.vector.tensor_mul(out=c_t, in0=s_t, in1=gamma_sb)
    return s_t, inv_t, c_t


def _emit_phase(nc, pools, nt, xc_tiles, yT, w_sb, bias_sb, inv, c_scale,
                rg_hook=None):
    """Quantize x with 1/s (DVE mult+max, min+addC; ACT subC->bf16), then
    per row-group run the transposed matmul (W^T blocks stationary, xi rows
    streaming, PSUM partition dim = out_features), fuse scale+bias into the
    ACT PSUM->SBUF copy, store y^T tiles. `inv` is a float or [P,1] tile;
    `c_scale` a [P,1] tile. Returns last emitted instruction per engine.

    Quantize chain (exact): clip-before-round equals round-then-clip since
    the bounds are integers, so
      t1 = max(x*inv, -127); t1 = min(t1, 127) + C; xi = bf16(t1 - C)
    yields round-half-even(clip(x/s)) exactly (+-1.5*2^23 trick; mult by a
    power of two is exact; integer results |.|<=127 are bf16-exact)."""
    t1_pool, xi_pool, yo_pool, ps_pool = pools
    last = {}
    for rg in range(nt):
        xc = xc_tiles[rg]
        xi_slices = []
        for h in range(2):
            t1 = t1_pool.tile([P, KC // 2, RSUB], F32, tag="t1")
            nc.vector.tensor_scalar(
                out=t1,
                in0=xc[:, h * (KC // 2): (h + 1) * (KC // 2), :],
                scalar1=inv,
                scalar2=-127.0,
                op0=mybir.AluOpType.mult,
                op1=mybir.AluOpType.max,
            )
            last["DVE"] = nc.vector.tensor_scalar(
                out=t1,
                in0=t1,
                scalar1=127.0,
                scalar2=ROUND_C,
                op0=mybir.AluOpType.min,
                op1=mybir.AluOpType.add,
            )
            for kk in range(KC // 2):
                xi = xi_pool.tile([P, RSUB], BF16, tag="xi")
                last["ACT"] = nc.scalar.activation(
                    out=xi,
                    in_=t1[:, kk, :],
                    func=mybir.ActivationFunctionType.Copy,
                    bias=-ROUND_C,
                    scale=1.0,
                )
                xi_slices.append(xi)
        if rg_hook is not None:
            rg_hook(rg, xc)
        for j in range(JC):
            ps = ps_pool.tile([P, RSUB], F32, tag="ps")
            for k in range(KC):
                last["PE"] = nc.tensor.matmul(
                    ps,
                    lhsT=w_sb[:, k, j * P: (j + 1) * P],
                    rhs=xi_slices[k],
                    start=(k == 0),
                    stop=(k == KC - 1),
                )
            yo = yo_pool.tile([P, RSUB], F32, tag="yo")
            last["ACT"] = nc.scalar.activation(
                out=yo,
                in_=ps,
                func=mybir.ActivationFunctionType.Identity,
                bias=bias_sb[:, j: j + 1],
                scale=c_scale,
            )
            last["POOL"] = nc.gpsimd.dma_start(
                out=yT[j * P: (j + 1) * P, rg * RSUB: (rg + 1) * RSUB],
                in_=yo,
            )
    return last


def build_program(rows: int = 4096, num_cores: int = N_CORES,
                  safe: bool = False) -> bacc.Bacc:
    """safe=False: fast program - speculative scale hardcoded, binade-check
    verdict exported to DRAM (host re-runs the safe program on mispredict).
    safe=True: self-contained exact program with an on-device If/redo."""
    assert rows % RSUB == 0
    nc = bacc.Bacc(
        "TRN2",
        target_bir_lowering=False,
        debug=False,
        enable_asserts=False,
        num_devices=num_cores,
    )
    nt = rows // RSUB
    # x shard pre-tiled on host: xt[t, p, c, r] = x[t*RSUB + r, c*P + p],
    # so every chunk load is fully contiguous.
    xt = nc.dram_tensor("xt", (nt, P, KC, RSUB), F32, kind="ExternalInput").ap()
    wq = nc.dram_tensor("wq", (IN_F, OUT_F), BF16, kind="ExternalInput").ap()
    # bias pre-transposed on host to [P, JC]: bqt[p, j] = bq[j*128 + p]
    bqt = nc.dram_tensor("bqt", (P, JC), F32, kind="ExternalInput").ap()
    gq = nc.dram_tensor("gq", (1, 1), F32, kind="ExternalInput").ap()
    # transposed output y^T [out_features, rows]; host transposes back
    yT = nc.dram_tensor("yT", (OUT_F, rows), F32, kind="ExternalOutput").ap()
    # binade-check verdict (int32 bits; >= 0 means speculation missed)
    chk_out = nc.dram_tensor("chk", (P, 1), I32, kind="ExternalOutput").ap()
    # Collectives cannot target I/O tensors; bounce through internal DRAM.
    cc_in = nc.dram_tensor("cc_in", (P, 1), F32).ap()
    cc_out = nc.dram_tensor("cc_out", (P, 1), F32).ap()
    # dummy collective to pre-warm the ncfw/collectives firmware
    ccw_in = nc.dram_tensor("ccw_in", (P, 1), F32).ap()
    ccw_out = nc.dram_tensor("ccw_out", (P, 1), F32).ap()

    with tile.TileContext(nc, num_cores=num_cores) as tc, ExitStack() as ctx:
        consts = ctx.enter_context(tc.tile_pool(name="consts", bufs=1))

        mask_t = consts.tile([P, 1], I32)
        nc.vector.memset(mask_t, -8388608)  # 0xFF800000: sign+exponent mask
        expc_t = consts.tile([P, 1], I32)
        nc.vector.memset(expc_t, 0x7F000000)  # bits of (254<<23)

        # All big loads ride the sync ring (its HWDGE queue shards across
        # all 16 SDMA engines; the scalar ring's does not). Interleave the
        # weight halves with chunk 0's halves so the first matmul's inputs
        # land together as early as possible.
        gamma_sb = consts.tile([P, 1], F32)
        w_sb = consts.tile([P, KC, OUT_F], BF16)
        bias_sb = consts.tile([P, JC], F32)
        w_src = wq.rearrange("(c p) o -> p c o", p=P)
        c_spec = consts.tile([P, 1], F32)
        partials = consts.tile([P, nt], F32)
        gmax_g = consts.tile([P, 1], F32)
        warm_rhs = consts.tile([P, RSUB], BF16)
        nc.vector.memset(warm_rhs, 0.0)
        warm_f = consts.tile([P, 1], F32)
        nc.vector.memset(warm_f, 1.0)
        warm_o = consts.tile([P, 1], F32)
        rg_cc = [list(range(num_cores))]

        with (
            tc.tile_pool(name="xc", bufs=nt) as xc_pool,
            tc.tile_pool(name="t1", bufs=2) as t1_pool,
            tc.tile_pool(name="xi", bufs=3 * KC) as xi_pool,
            tc.tile_pool(name="yo", bufs=6) as yo_pool,
            tc.tile_pool(name="ps", bufs=8, space="PSUM") as ps_pool,
        ):
            pools = (t1_pool, xi_pool, yo_pool, ps_pool)

            # eager x loads: all chunks issued up front on the sync ring,
            # two half-chunk DMAs per chunk (finer landing granularity).
            # Issue order front-loads what the first matmul needs:
            # w half 0, chunk0 half 0, w half 1, chunk0 half 1, consts,
            # then chunks 1..nt-1.
            xc_tiles = []
            for _t in range(nt):
                xc = xc_pool.tile([P, KC, RSUB], F32, tag="xc")
                xc_tiles.append(xc)

            def load_half(t, h):
                return nc.sync.dma_start(
                    out=xc_tiles[t][:, h * (KC // 2): (h + 1) * (KC // 2), :],
                    in_=xt[t, :, h * (KC // 2): (h + 1) * (KC // 2), :],
                )

            nc.sync.dma_start(out=w_sb[:, 0: KC // 2, :],
                              in_=w_src[:, 0: KC // 2, :])
            load_half(0, 0)
            nc.sync.dma_start(out=w_sb[:, KC // 2: KC, :],
                              in_=w_src[:, KC // 2: KC, :])
            load_half(0, 1)
            nc.sync.dma_start(out=gamma_sb, in_=gq.to_broadcast((P, 1)))
            nc.sync.dma_start(out=bias_sb, in_=bqt)
            last_sp = None
            for t in range(1, nt):
                for h in range(2):
                    last_sp = load_half(t, h)

            # c = s_spec * gamma for the speculative fast path
            nc.vector.tensor_scalar(
                out=c_spec, in0=gamma_sb, scalar1=S_SPEC, scalar2=None,
                op0=mybir.AluOpType.mult,
            )

            # --- warmups, all dependency-free ---
            # PE: junk matmuls flip HAM to full clock before the first
            # real matmul
            warm_ps = ps_pool.tile([P, RSUB], F32, tag="ps")
            for _ in range(10):
                nc.tensor.matmul(
                    warm_ps, lhsT=warm_rhs[:, 0:P], rhs=warm_rhs,
                    start=True, stop=True,
                )
            # gpsimd: pre-load the Q7 partition-reduce library
            nc.gpsimd.partition_all_reduce(
                warm_o, warm_f, channels=P, reduce_op=bass_isa.ReduceOp.max,
            )
            # collectives firmware: dummy 512B AllReduce so the real one
            # later starts without the ncfw cold-wake latency
            nc.gpsimd.dma_start(out=ccw_in, in_=warm_f)
            nc.gpsimd.collective_compute(
                "AllReduce", mybir.AluOpType.max, replica_groups=rg_cc,
                ins=[ccw_in.opt()], outs=[ccw_out.opt()],
            )

            def rg_hook(rg, xc):
                # per-chunk absmax, off the critical path (reads the
                # resident x chunk); placed in the DVE stream inside rg's
                # segment so it never heads-of-line-blocks on a load
                nc.vector.tensor_reduce(
                    out=partials[:, rg: rg + 1],
                    in_=xc,
                    axis=mybir.AxisListType.XY,
                    op=mybir.AluOpType.max,
                    apply_absolute_value=True,
                )
                if rg == nt - 1:
                    # full local max -> 512B AllReduce(max) across the 8
                    # cores; completes well before the last matmul
                    lmax = consts.tile([P, 1], F32)
                    nc.vector.tensor_reduce(
                        out=lmax, in_=partials, axis=mybir.AxisListType.X,
                        op=mybir.AluOpType.max,
                    )
                    gmax_l = consts.tile([P, 1], F32)
                    nc.gpsimd.partition_all_reduce(
                        gmax_l, lmax, channels=P,
                        reduce_op=bass_isa.ReduceOp.max,
                    )
                    nc.gpsimd.dma_start(out=cc_in, in_=gmax_l)
                    nc.gpsimd.collective_compute(
                        "AllReduce", mybir.AluOpType.max, replica_groups=rg_cc,
                        ins=[cc_in.opt()], outs=[cc_out.opt()],
                    )
                    nc.gpsimd.dma_start(out=gmax_g, in_=cc_out)

            last = _emit_phase(
                nc, pools, nt, xc_tiles, yT, w_sb, bias_sb,
                INV_SPEC, c_spec, rg_hook=rg_hook,
            )
            last["SP"] = last_sp

            # --- speculation check: s_spec is the true scale iff
            # v = gmax/127+eps lies in [s_spec, 2*s_spec), i.e.
            # sign(v - s) != sign(v - 2s). XOR of the float bits makes the
            # pass condition a single sign test: chk < 0 <=> in-binade. ---
            v_g = consts.tile([P, 1], F32)
            nc.vector.tensor_scalar(
                out=v_g,
                in0=gmax_g,
                scalar1=float(np.float32(1.0 / 127.0)),
                scalar2=float(np.float32(EPS)),
                op0=mybir.AluOpType.mult,
                op1=mybir.AluOpType.add,
            )
            a_t = consts.tile([P, 1], F32)
            nc.vector.tensor_scalar(
                out=a_t, in0=v_g, scalar1=-S_SPEC, scalar2=None,
                op0=mybir.AluOpType.add,
            )
            b_t = consts.tile([P, 1], F32)
            nc.vector.tensor_scalar(
                out=b_t, in0=v_g, scalar1=-2.0 * S_SPEC, scalar2=None,
                op0=mybir.AluOpType.add,
            )
            chk = consts.tile([P, 1], I32)
            last["DVE"] = nc.vector.tensor_tensor(
                out=chk, in0=a_t.bitcast(I32), in1=b_t.bitcast(I32),
                op=mybir.AluOpType.bitwise_xor,
            )
            if not safe:
                # fast program: export the verdict; the host re-runs the
                # safe program in the (P < 1e-7) mispredict case. No
                # on-device branch means no skipped-region semaphore
                # reconciliation in the epilogue.
                nc.gpsimd.dma_start(out=chk_out, in_=chk)
            else:
                nc.gpsimd.dma_start(out=chk_out, in_=chk)
                regs = nc.alloc_registers(
                    "spec_chk",
                    bass.OrderedSet([
                        mybir.EngineType.SP,
                        mybir.EngineType.DVE,
                        mybir.EngineType.Activation,
                        mybir.EngineType.PE,
                        mybir.EngineType.Pool,
                    ]),
                )
                # Pin each engine's reg_load after its last speculative-
                # phase instruction: the load waits on the AllReduce, and
                # the Tile scheduler would otherwise be free to place it
                # mid-stream, stalling that engine's FIFO on the collective.
                eng_key = {
                    mybir.EngineType.PE: "PE",
                    mybir.EngineType.DVE: "DVE",
                    mybir.EngineType.Activation: "ACT",
                    mybir.EngineType.SP: "SP",
                    mybir.EngineType.Pool: "POOL",
                }
                for reg in regs:
                    ld = nc.reg_load(reg, chk[0:1, 0:1])
                    prev = last.get(eng_key[reg.engine])
                    if prev is not None:
                        tile.add_dep_helper(
                            ld.ins, prev.ins, sync=False,
                            reason="speculation check after spec phase",
                        )
                # negative iff inside the speculated binade (fast path)
                with tc.If(nc.snap(regs) >= 0):
                    # mismatch: redo with the exact global scale
                    # (x chunks are still resident in SBUF - no reloads)
                    _, inv_g, c_g = _emit_scale_chain(
                        nc, consts, gmax_g, gamma_sb, mask_t, expc_t, "g")
                    _emit_phase(nc, pools, nt, xc_tiles, yT, w_sb, bias_sb,
                                inv_g, c_g)

    nc.compile()
    return nc


def quantize_params(weight: np.ndarray, bias: np.ndarray):
    """Ternary-quantize weight/bias exactly as the reference (f64 math whose
    f32 rounding matches jax-f32; verified margins are orders of magnitude
    above f32 accumulation differences)."""
    w64 = weight.astype(np.float64)
    g_w = np.float32(np.abs(w64).mean())
    wi = np.clip(np.round(w64 / (np.float64(g_w) + EPS)), -1.0, 1.0)
    b64 = bias.astype(np.float64)
    g_b = np.float32(np.abs(b64).mean())
    bi = np.clip(np.round(b64 / (np.float64(g_b) + EPS)), -1.0, 1.0)
    bq = (bi * np.float64(g_b)).astype(np.float32)  # exact: {-g_b, 0, g_b}
    return wi, g_w, bq


_PROGRAM_CACHE: dict[tuple[int, bool], bacc.Bacc] = {}


def _get_program(rows: int, safe: bool = False) -> bacc.Bacc:
    key = (rows, safe)
    if key not in _PROGRAM_CACHE:
        _PROGRAM_CACHE[key] = build_program(rows, safe=safe)
    return _PROGRAM_CACHE[key]


def tile_x_shard(x2d: np.ndarray) -> np.ndarray:
    """[rows, IN_F] -> [nt, P, KC, RSUB] with xt[t,p,c,r] = x[t*RSUB+r, c*P+p]."""
    rows = x2d.shape[0]
    return np.ascontiguousarray(
        x2d.reshape(rows // RSUB, RSUB, KC, P).transpose(0, 3, 2, 1)
    )


def prepare_in_maps(x: np.ndarray, weight: np.ndarray, bias: np.ndarray):
    x = np.asarray(x, dtype=np.float32)
    weight = np.asarray(weight, dtype=np.float32)
    bias = np.asarray(bias, dtype=np.float32)
    batch, rows, in_f = x.shape
    assert batch == N_CORES and in_f == IN_F and weight.shape == (OUT_F, IN_F)

    wi, g_w, bq = quantize_params(weight, bias)
    wq_t = np.ascontiguousarray(wi.T).astype(ml_dtypes.bfloat16)  # [in, out]
    bqt = np.ascontiguousarray(bq.reshape(JC, P).T)               # [P, JC]
    gq = np.array([[g_w]], dtype=np.float32)

    in_maps = []
    for c in range(N_CORES):
        in_maps.append(
            {
                "xt": tile_x_shard(x[c]),
                "wq": wq_t,
                "bqt": bqt,
                "gq": gq,
            }
        )
    return in_maps, rows


def kernel(x: np.ndarray, weight: np.ndarray, bias: np.ndarray) -> np.ndarray:
    in_maps, rows = prepare_in_maps(x, weight, bias)
    nc = _get_program(rows)
    res = bass_utils.run_bass_kernel_spmd(nc, in_maps, core_ids=list(range(N_CORES)))
    # device-computed binade check: int32 bits of the XOR sign test are
    # negative iff the speculated scale is the true global scale
    if any(int(res.results[c]["chk"][0, 0]) >= 0 for c in range(N_CORES)):
        # speculation missed (P < 1e-7 for randn inputs): run the
        # self-contained exact program with the on-device redo branch
        nc_safe = _get_program(rows, safe=True)
        res = bass_utils.run_bass_kernel_spmd(
            nc_safe, in_maps, core_ids=list(range(N_CORES)))
    return np.stack(
        [np.ascontiguousarray(res.results[c]["yT"].T) for c in range(N_CORES)],
        axis=0,
    )


# revision 16
# speedup vs baseline: 1.0862x; 1.0482x over previous
"""BitNetLinear Trainium2 kernel (8 NeuronCores, SPMD data-parallel).

y = round(clip(x, +-127*s)/s)*s @ (ternary(W))^T + ternary(b)
with s = exp2(floor(log2(max|x|/127 + eps))) a power of two (global over x).

Sharding: batch dim (8) -> one batch of [4096, 1024] per core.

v2 design (vs the 208us v1):
 * TRANSPOSED GEMM: compute y^T with out_features on PSUM partitions
   (lhsT = W^T block stationary, quantized-x rows streaming). The ternary
   bias becomes per-partition and fuses into the ACT-engine PSUM->SBUF
   copy (activation Identity: out = ps*c + b) - no DVE bias pass, stores
   depend only on PE+ACT. Host transposes y back (layout only).
 * HARDCODED SPECULATIVE SCALE: for this input regime (randn) the global
   power-of-two scale is 2^-5 with overwhelming probability
   (P(other binade) < 1e-7 for any randn(0,1) of this size). The kernel
   runs the whole pipeline with s_spec = 2^-5 baked in, so the first
   matmul starts as soon as chunk 0 lands - no absmax on the critical
   path. Exactness is unconditional: per-chunk absmaxes accumulate off
   the critical path, a 512B AllReduce(max) produces the true global
   max, and a one-instruction binade check (sign(v-s) XOR sign(v-2s))
   branches to an exact full redo with the device-computed scale if the
   speculation missed (ANY input remains bit-correct, just slower).
 * EAGER x RESIDENCY: all 16 MiB of the x shard is loaded up front into
   SBUF (it fits), so loads never pace compute, the AllReduce finishes
   long before the last matmul, and the redo path (if taken) reads x
   from SBUF without reloading.

x is quantized to integer-valued bf16 (round-half-even via the +-1.5*2^23
trick); the bf16 matmul with fp32 PSUM accumulation is exact integer
arithmetic (|x_int| <= 127, w in {-1,0,1}, |acc| < 2^24); the result is
scaled by s*gamma_w and the ternary bias added, all in the ACT copy.
"""

import numpy as np
import ml_dtypes
from contextlib import ExitStack

import concourse.bass as bass
import concourse.mybir as mybir
import concourse.tile as tile
from concourse import bacc, bass_isa, bass_utils

F32 = mybir.dt.float32
BF16 = mybir.dt.bfloat16
I32 = mybir.dt.int32

N_CORES = 8
P = 128
IN_F = 1024
OUT_F = 1024
KC = IN_F // P          # 8 contraction chunks of 128
JC = OUT_F // P         # 8 output blocks of 128
RSUB = 512              # rows per chunk / row-group
ROUND_C = 12582912.0    # 1.5 * 2**23: float32 round-half-even trick
EPS = 1e-8
S_SPEC = 2.0 ** -5      # speculative global scale (binade of max|x|/127+eps)
INV_SPEC = 2.0 ** 5


def _emit_scale_chain(nc, consts, gmax, gamma_sb, mask_t, expc_t, tag):
    """From a [P,1] absmax tile, compute s = exp2(floor(log2(m/127+eps)))
    via exponent masking, 1/s via exponent arithmetic, and c = s*gamma."""
    v_t = consts.tile([P, 1], F32, tag=f"v_{tag}")
    nc.vector.tensor_scalar(
        out=v_t,
        in0=gmax,
        scalar1=float(np.float32(1.0 / 127.0)),
        scalar2=float(np.float32(EPS)),
        op0=mybir.AluOpType.mult,
        op1=mybir.AluOpType.add,
    )
    s_t = consts.tile([P, 1], F32, tag=f"s_{tag}")
    nc.vector.tensor_tensor(
        out=s_t.bitcast(I32),
        in0=v_t.bitcast(I32),
        in1=mask_t,
        op=mybir.AluOpType.bitwise_and,
    )
    inv_t = consts.tile([P, 1], F32, tag=f"inv_{tag}")
    nc.vector.tensor_tensor(
        out=inv_t.bitcast(I32),
        in0=expc_t,
        in1=s_t.bitcast(I32),
        op=mybir.AluOpType.subtract,
    )
    c_t = consts.tile([P, 1], F32, tag=f"c_{tag}")
    nc.vector.tensor_mul(out=c_t, in0=s_t, in1=gamma_sb)
    return s_t, inv_t, c_t


def _emit_phase(nc, pools, nt, xc_tiles, yT, w_sb, bias_sb, inv, c_scale,
                rg_hook=None, store_engine=None, split_rg0=False):
    """Quantize x with 1/s (DVE mult+max, min+addC; ACT subC->bf16), then
    per row-group run the transposed matmul (W^T blocks stationary, xi rows
    streaming, PSUM partition dim = out_features), fuse scale+bias into the
    ACT PSUM->SBUF copy, store y^T tiles. `inv` is a float or [P,1] tile;
    `c_scale` a [P,1] tile. Returns last emitted instruction per engine.

    Stores go out on `store_engine` (default nc.gpsimd). The sync/HWDGE
    ring is preferred for the fast phase: SWDGE costs ~2us of Q7
    descriptor generation per dma, which backs up 64 stores and stalls
    the ACT copies on yo-pool recycling.

    rg_hook(rg, xc) is called one row-group LATE (after rg+1's quantize
    ops) so its absmax reduce never delays the next group's xi production.

    split_rg0 runs row-group 0's matmuls as two k-sweeps (k 0..3 then
    4..7, PSUM accumulation group held open across the sweeps) so the
    first 32 matmuls depend only on the first halves of w_sb and chunk 0.

    Quantize chain (exact): clip-before-round equals round-then-clip since
    the bounds are integers, so
      t1 = max(x*inv, -127); t1 = min(t1, 127) + C; xi = bf16(t1 - C)
    yields round-half-even(clip(x/s)) exactly (+-1.5*2^23 trick; mult by a
    power of two is exact; integer results |.|<=127 are bf16-exact)."""
    t1_pool, xi_pool, yo_pool, ps_pool = pools
    if store_engine is None:
        store_engine = nc.gpsimd
    last = {}

    def emit_quant(rg):
        xc = xc_tiles[rg]
        xi_slices = []
        for h in range(2):
            t1 = t1_pool.tile([P, KC // 2, RSUB], F32, tag="t1")
            nc.vector.tensor_scalar(
                out=t1,
                in0=xc[:, h * (KC // 2): (h + 1) * (KC // 2), :],
                scalar1=inv,
                scalar2=-127.0,
                op0=mybir.AluOpType.mult,
                op1=mybir.AluOpType.max,
            )
            last["DVE"] = nc.vector.tensor_scalar(
                out=t1,
                in0=t1,
                scalar1=127.0,
                scalar2=ROUND_C,
                op0=mybir.AluOpType.min,
                op1=mybir.AluOpType.add,
            )
            for kk in range(KC // 2):
                xi = xi_pool.tile([P, RSUB], BF16, tag="xi")
                last["ACT"] = nc.scalar.activation(
                    out=xi,
                    in_=t1[:, kk, :],
                    func=mybir.ActivationFunctionType.Copy,
                    bias=-ROUND_C,
                    scale=1.0,
                )
                xi_slices.append(xi)
        return xi_slices

    def emit_mm_out(rg, xi_slices):
        if split_rg0 and rg == 0:
            ps_tiles = []
            for j in range(JC):
                ps = ps_pool.tile([P, RSUB], F32, tag="ps")
                ps_tiles.append(ps)
            for ka, kb, st, sp in ((0, KC // 2, True, False),
                                   (KC // 2, KC, False, True)):
                for j in range(JC):
                    for k in range(ka, kb):
                        last["PE"] = nc.tensor.matmul(
                            ps_tiles[j],
                            lhsT=w_sb[:, k, j * P: (j + 1) * P],
                            rhs=xi_slices[k],
                            start=st and (k == ka),
                            stop=sp and (k == kb - 1),
                            skip_group_check=True,
                        )
            for j in range(JC):
                yo = yo_pool.tile([P, RSUB], F32, tag="yo")
                last["ACT"] = nc.scalar.activation(
                    out=yo,
                    in_=ps_tiles[j],
                    func=mybir.ActivationFunctionType.Identity,
                    bias=bias_sb[:, j: j + 1],
                    scale=c_scale,
                )
                last["ST"] = store_engine.dma_start(
                    out=yT[j * P: (j + 1) * P, rg * RSUB: (rg + 1) * RSUB],
                    in_=yo,
                )
            return
        for j in range(JC):
            ps = ps_pool.tile([P, RSUB], F32, tag="ps")
            for k in range(KC):
                last["PE"] = nc.tensor.matmul(
                    ps,
                    lhsT=w_sb[:, k, j * P: (j + 1) * P],
                    rhs=xi_slices[k],
                    start=(k == 0),
                    stop=(k == KC - 1),
                )
            yo = yo_pool.tile([P, RSUB], F32, tag="yo")
            last["ACT"] = nc.scalar.activation(
                out=yo,
                in_=ps,
                func=mybir.ActivationFunctionType.Identity,
                bias=bias_sb[:, j: j + 1],
                scale=c_scale,
            )
            last["ST"] = store_engine.dma_start(
                out=yT[j * P: (j + 1) * P, rg * RSUB: (rg + 1) * RSUB],
                in_=yo,
            )

    for rg in range(nt):
        xi_slices = emit_quant(rg)
        # lagged hook: rg-1's absmax goes after rg's quantize in the DVE
        # stream, keeping it off the xi critical path
        if rg_hook is not None and rg > 0:
            rg_hook(rg - 1, xc_tiles[rg - 1])
        emit_mm_out(rg, xi_slices)
    if rg_hook is not None:
        rg_hook(nt - 1, xc_tiles[nt - 1])
    return last


def build_program(rows: int = 4096, num_cores: int = N_CORES,
                  safe: bool = False) -> bacc.Bacc:
    """safe=False: fast program - speculative scale hardcoded, binade-check
    verdict exported to DRAM (host re-runs the safe program on mispredict).
    safe=True: self-contained exact program with an on-device If/redo."""
    assert rows % RSUB == 0
    nc = bacc.Bacc(
        "TRN2",
        target_bir_lowering=False,
        debug=False,
        enable_asserts=False,
        num_devices=num_cores,
    )
    nt = rows // RSUB
    # x shard pre-tiled on host: xt[t, p, c, r] = x[t*RSUB + r, c*P + p],
    # so every chunk load is fully contiguous.
    xt = nc.dram_tensor("xt", (nt, P, KC, RSUB), F32, kind="ExternalInput").ap()
    wq = nc.dram_tensor("wq", (IN_F, OUT_F), BF16, kind="ExternalInput").ap()
    # bias pre-transposed on host to [P, JC]: bqt[p, j] = bq[j*128 + p]
    bqt = nc.dram_tensor("bqt", (P, JC), F32, kind="ExternalInput").ap()
    gq = nc.dram_tensor("gq", (1, 1), F32, kind="ExternalInput").ap()
    # transposed output y^T [out_features, rows]; host transposes back
    yT = nc.dram_tensor("yT", (OUT_F, rows), F32, kind="ExternalOutput").ap()
    # binade-check verdict (int32 bits; >= 0 means speculation missed)
    chk_out = nc.dram_tensor("chk", (P, 1), I32, kind="ExternalOutput").ap()
    # Collectives cannot target I/O tensors; bounce through internal DRAM.
    cc_in = nc.dram_tensor("cc_in", (P, 1), F32).ap()
    cc_out = nc.dram_tensor("cc_out", (P, 1), F32).ap()
    # dummy collective to pre-warm the ncfw/collectives firmware
    ccw_in = nc.dram_tensor("ccw_in", (P, 1), F32).ap()
    ccw_out = nc.dram_tensor("ccw_out", (P, 1), F32).ap()

    with tile.TileContext(nc, num_cores=num_cores) as tc, ExitStack() as ctx:
        consts = ctx.enter_context(tc.tile_pool(name="consts", bufs=1))

        mask_t = consts.tile([P, 1], I32)
        nc.vector.memset(mask_t, -8388608)  # 0xFF800000: sign+exponent mask
        expc_t = consts.tile([P, 1], I32)
        nc.vector.memset(expc_t, 0x7F000000)  # bits of (254<<23)

        # All big loads ride the sync ring (its HWDGE queue shards across
        # all 16 SDMA engines; the scalar ring's does not). Interleave the
        # weight halves with chunk 0's halves so the first matmul's inputs
        # land together as early as possible.
        gamma_sb = consts.tile([P, 1], F32)
        w_sb = consts.tile([P, KC, OUT_F], BF16)
        bias_sb = consts.tile([P, JC], F32)
        w_src = wq.rearrange("(c p) o -> p c o", p=P)
        c_spec = consts.tile([P, 1], F32)
        partials = consts.tile([P, nt], F32)
        gmax_g = consts.tile([P, 1], F32)
        warm_rhs = consts.tile([P, RSUB], BF16)
        nc.vector.memset(warm_rhs, 0.0)
        warm_f = consts.tile([P, 1], F32)
        nc.vector.memset(warm_f, 1.0)
        warm_o = consts.tile([P, 1], F32)
        rg_cc = [list(range(num_cores))]

        with (
            tc.tile_pool(name="xc", bufs=nt) as xc_pool,
            tc.tile_pool(name="t1", bufs=2) as t1_pool,
            tc.tile_pool(name="xi", bufs=3 * KC) as xi_pool,
            tc.tile_pool(name="yo", bufs=10) as yo_pool,
            tc.tile_pool(name="ps", bufs=8, space="PSUM") as ps_pool,
        ):
            pools = (t1_pool, xi_pool, yo_pool, ps_pool)

            # eager x loads: all chunks issued up front on the sync ring,
            # two half-chunk DMAs per chunk (finer landing granularity).
            # Issue order front-loads what the first matmul needs:
            # w half 0, chunk0 half 0, w half 1, chunk0 half 1, consts,
            # then chunks 1..nt-1.
            xc_tiles = []
            for _t in range(nt):
                xc = xc_pool.tile([P, KC, RSUB], F32, tag="xc")
                xc_tiles.append(xc)

            def load_half(t, h):
                return nc.sync.dma_start(
                    out=xc_tiles[t][:, h * (KC // 2): (h + 1) * (KC // 2), :],
                    in_=xt[t, :, h * (KC // 2): (h + 1) * (KC // 2), :],
                )

            nc.sync.dma_start(out=gamma_sb, in_=gq.to_broadcast((P, 1)))
            nc.sync.dma_start(out=w_sb[:, 0: KC // 2, :],
                              in_=w_src[:, 0: KC // 2, :])
            load_half(0, 0)
            nc.sync.dma_start(out=w_sb[:, KC // 2: KC, :],
                              in_=w_src[:, KC // 2: KC, :])
            load_half(0, 1)
            nc.sync.dma_start(out=bias_sb, in_=bqt)
            for t in range(1, nt):
                for h in range(2):
                    load_half(t, h)

            # c = s_spec * gamma for the speculative fast path
            nc.vector.tensor_scalar(
                out=c_spec, in0=gamma_sb, scalar1=S_SPEC, scalar2=None,
                op0=mybir.AluOpType.mult,
            )

            # --- warmups, all dependency-free ---
            # PE: junk matmuls flip HAM to full clock before the first
            # real matmul
            warm_ps = ps_pool.tile([P, RSUB], F32, tag="ps")
            for _ in range(10):
                nc.tensor.matmul(
                    warm_ps, lhsT=warm_rhs[:, 0:P], rhs=warm_rhs,
                    start=True, stop=True,
                )
            # gpsimd: pre-load the Q7 partition-reduce library
            nc.gpsimd.partition_all_reduce(
                warm_o, warm_f, channels=P, reduce_op=bass_isa.ReduceOp.max,
            )
            # collectives firmware: dummy 512B AllReduce so the real one
            # later starts without the ncfw cold-wake latency
            nc.gpsimd.dma_start(out=ccw_in, in_=warm_f)
            nc.gpsimd.collective_compute(
                "AllReduce", mybir.AluOpType.max, replica_groups=rg_cc,
                ins=[ccw_in.opt()], outs=[ccw_out.opt()],
            )

            def rg_hook(rg, xc):
                # per-chunk absmax, off the critical path (reads the
                # resident x chunk); placed in the DVE stream inside rg's
                # segment so it never heads-of-line-blocks on a load
                nc.vector.tensor_reduce(
                    out=partials[:, rg: rg + 1],
                    in_=xc,
                    axis=mybir.AxisListType.XY,
                    op=mybir.AluOpType.max,
                    apply_absolute_value=True,
                )
                if rg == nt - 1:
                    # full local max -> 512B AllReduce(max) across the 8
                    # cores; completes well before the last matmul
                    lmax = consts.tile([P, 1], F32)
                    nc.vector.tensor_reduce(
                        out=lmax, in_=partials, axis=mybir.AxisListType.X,
                        op=mybir.AluOpType.max,
                    )
                    gmax_l = consts.tile([P, 1], F32)
                    nc.gpsimd.partition_all_reduce(
                        gmax_l, lmax, channels=P,
                        reduce_op=bass_isa.ReduceOp.max,
                    )
                    nc.gpsimd.dma_start(out=cc_in, in_=gmax_l)
                    nc.gpsimd.collective_compute(
                        "AllReduce", mybir.AluOpType.max, replica_groups=rg_cc,
                        ins=[cc_in.opt()], outs=[cc_out.opt()],
                    )
                    nc.gpsimd.dma_start(out=gmax_g, in_=cc_out)

            last = _emit_phase(
                nc, pools, nt, xc_tiles, yT, w_sb, bias_sb,
                INV_SPEC, c_spec, rg_hook=rg_hook,
                store_engine=nc.sync, split_rg0=True,
            )

            # --- speculation check: s_spec is the true scale iff
            # v = gmax/127+eps lies in [s_spec, 2*s_spec), i.e.
            # sign(v - s) != sign(v - 2s). XOR of the float bits makes the
            # pass condition a single sign test: chk < 0 <=> in-binade. ---
            v_g = consts.tile([P, 1], F32)
            nc.vector.tensor_scalar(
                out=v_g,
                in0=gmax_g,
                scalar1=float(np.float32(1.0 / 127.0)),
                scalar2=float(np.float32(EPS)),
                op0=mybir.AluOpType.mult,
                op1=mybir.AluOpType.add,
            )
            a_t = consts.tile([P, 1], F32)
            nc.vector.tensor_scalar(
                out=a_t, in0=v_g, scalar1=-S_SPEC, scalar2=None,
                op0=mybir.AluOpType.add,
            )
            b_t = consts.tile([P, 1], F32)
            nc.vector.tensor_scalar(
                out=b_t, in0=v_g, scalar1=-2.0 * S_SPEC, scalar2=None,
                op0=mybir.AluOpType.add,
            )
            chk = consts.tile([P, 1], I32)
            last["DVE"] = nc.vector.tensor_tensor(
                out=chk, in0=a_t.bitcast(I32), in1=b_t.bitcast(I32),
                op=mybir.AluOpType.bitwise_xor,
            )
            if not safe:
                # fast program: export the verdict; the host re-runs the
                # safe program in the (P < 1e-7) mispredict case. No
                # on-device branch means no skipped-region semaphore
                # reconciliation in the epilogue.
                nc.gpsimd.dma_start(out=chk_out, in_=chk)
            else:
                last["POOL"] = nc.gpsimd.dma_start(out=chk_out, in_=chk)
                regs = nc.alloc_registers(
                    "spec_chk",
                    bass.OrderedSet([
                        mybir.EngineType.SP,
                        mybir.EngineType.DVE,
                        mybir.EngineType.Activation,
                        mybir.EngineType.PE,
                        mybir.EngineType.Pool,
                    ]),
                )
                # Pin each engine's reg_load after its last speculative-
                # phase instruction: the load waits on the AllReduce, and
                # the Tile scheduler would otherwise be free to place it
                # mid-stream, stalling that engine's FIFO on the collective.
                eng_key = {
                    mybir.EngineType.PE: "PE",
                    mybir.EngineType.DVE: "DVE",
                    mybir.EngineType.Activation: "ACT",
                    mybir.EngineType.SP: "ST",
                    mybir.EngineType.Pool: "POOL",
                }
                for reg in regs:
                    ld = nc.reg_load(reg, chk[0:1, 0:1])
                    prev = last.get(eng_key[reg.engine])
                    if prev is not None:
                        tile.add_dep_helper(
                            ld.ins, prev.ins, sync=False,
                            reason="speculation check after spec phase",
                        )
                # negative iff inside the speculated binade (fast path)
                with tc.If(nc.snap(regs) >= 0):
                    # mismatch: redo with the exact global scale
                    # (x chunks are still resident in SBUF - no reloads)
                    _, inv_g, c_g = _emit_scale_chain(
                        nc, consts, gmax_g, gamma_sb, mask_t, expc_t, "g")
                    _emit_phase(nc, pools, nt, xc_tiles, yT, w_sb, bias_sb,
                                inv_g, c_g, store_engine=nc.sync)

    nc.compile()
    return nc


def quantize_params(weight: np.ndarray, bias: np.ndarray):
    """Ternary-quantize weight/bias exactly as the reference (f64 math whose
    f32 rounding matches jax-f32; verified margins are orders of magnitude
    above f32 accumulation differences)."""
    w64 = weight.astype(np.float64)
    g_w = np.float32(np.abs(w64).mean())
    wi = np.clip(np.round(w64 / (np.float64(g_w) + EPS)), -1.0, 1.0)
    b64 = bias.astype(np.float64)
    g_b = np.float32(np.abs(b64).mean())
    bi = np.clip(np.round(b64 / (np.float64(g_b) + EPS)), -1.0, 1.0)
    bq = (bi * np.float64(g_b)).astype(np.float32)  # exact: {-g_b, 0, g_b}
    return wi, g_w, bq


_PROGRAM_CACHE: dict[tuple[int, bool], bacc.Bacc] = {}


def _get_program(rows: int, safe: bool = False) -> bacc.Bacc:
    key = (rows, safe)
    if key not in _PROGRAM_CACHE:
        _PROGRAM_CACHE[key] = build_program(rows, safe=safe)
    return _PROGRAM_CACHE[key]


def tile_x_shard(x2d: np.ndarray) -> np.ndarray:
    """[rows, IN_F] -> [nt, P, KC, RSUB] with xt[t,p,c,r] = x[t*RSUB+r, c*P+p]."""
    rows = x2d.shape[0]
    return np.ascontiguousarray(
        x2d.reshape(rows // RSUB, RSUB, KC, P).transpose(0, 3, 2, 1)
    )


def prepare_in_maps(x: np.ndarray, weight: np.ndarray, bias: np.ndarray):
    x = np.asarray(x, dtype=np.float32)
    weight = np.asarray(weight, dtype=np.float32)
    bias = np.asarray(bias, dtype=np.float32)
    batch, rows, in_f = x.shape
    assert batch == N_CORES and in_f == IN_F and weight.shape == (OUT_F, IN_F)

    wi, g_w, bq = quantize_params(weight, bias)
    wq_t = np.ascontiguousarray(wi.T).astype(ml_dtypes.bfloat16)  # [in, out]
    bqt = np.ascontiguousarray(bq.reshape(JC, P).T)               # [P, JC]
    gq = np.array([[g_w]], dtype=np.float32)

    in_maps = []
    for c in range(N_CORES):
        in_maps.append(
            {
                "xt": tile_x_shard(x[c]),
                "wq": wq_t,
                "bqt": bqt,
                "gq": gq,
            }
        )
    return in_maps, rows


def kernel(x: np.ndarray, weight: np.ndarray, bias: np.ndarray) -> np.ndarray:
    in_maps, rows = prepare_in_maps(x, weight, bias)
    nc = _get_program(rows)
    res = bass_utils.run_bass_kernel_spmd(nc, in_maps, core_ids=list(range(N_CORES)))
    # device-computed binade check: int32 bits of the XOR sign test are
    # negative iff the speculated scale is the true global scale
    if any(int(res.results[c]["chk"][0, 0]) >= 0 for c in range(N_CORES)):
        # speculation missed (P < 1e-7 for randn inputs): run the
        # self-contained exact program with the on-device redo branch
        nc_safe = _get_program(rows, safe=True)
        res = bass_utils.run_bass_kernel_spmd(
            nc_safe, in_maps, core_ids=list(range(N_CORES)))
    return np.stack(
        [np.ascontiguousarray(res.results[c]["yT"].T) for c in range(N_CORES)],
        axis=0,
    )


# revision 19
# speedup vs baseline: 1.0950x; 1.0081x over previous
"""BitNetLinear Trainium2 kernel (8 NeuronCores, SPMD data-parallel).

y = round(clip(x, +-127*s)/s)*s @ (ternary(W))^T + ternary(b)
with s = exp2(floor(log2(max|x|/127 + eps))) a power of two (global over x).

Sharding: batch dim (8) -> one batch of [4096, 1024] per core.

v2 design (vs the 208us v1):
 * TRANSPOSED GEMM: compute y^T with out_features on PSUM partitions
   (lhsT = W^T block stationary, quantized-x rows streaming). The ternary
   bias becomes per-partition and fuses into the ACT-engine PSUM->SBUF
   copy (activation Identity: out = ps*c + b) - no DVE bias pass, stores
   depend only on PE+ACT. Host transposes y back (layout only).
 * HARDCODED SPECULATIVE SCALE: for this input regime (randn) the global
   power-of-two scale is 2^-5 with overwhelming probability
   (P(other binade) < 1e-7 for any randn(0,1) of this size). The kernel
   runs the whole pipeline with s_spec = 2^-5 baked in, so the first
   matmul starts as soon as chunk 0 lands - no absmax on the critical
   path. Exactness is unconditional: per-chunk absmaxes accumulate off
   the critical path, a 512B AllReduce(max) produces the true global
   max, and a one-instruction binade check (sign(v-s) XOR sign(v-2s))
   branches to an exact full redo with the device-computed scale if the
   speculation missed (ANY input remains bit-correct, just slower).
 * EAGER x RESIDENCY: all 16 MiB of the x shard is loaded up front into
   SBUF (it fits), so loads never pace compute, the AllReduce finishes
   long before the last matmul, and the redo path (if taken) reads x
   from SBUF without reloading.

x is quantized to integer-valued bf16 (round-half-even via the +-1.5*2^23
trick); the bf16 matmul with fp32 PSUM accumulation is exact integer
arithmetic (|x_int| <= 127, w in {-1,0,1}, |acc| < 2^24); the result is
scaled by s*gamma_w and the ternary bias added, all in the ACT copy.
"""

import numpy as np
import ml_dtypes
from contextlib import ExitStack

import concourse.bass as bass
import concourse.mybir as mybir
import concourse.tile as tile
from concourse import bacc, bass_isa, bass_utils

F32 = mybir.dt.float32
BF16 = mybir.dt.bfloat16
I32 = mybir.dt.int32

N_CORES = 8
P = 128
IN_F = 1024
OUT_F = 1024
KC = IN_F // P          # 8 contraction chunks of 128
JC = OUT_F // P         # 8 output blocks of 128
RSUB = 512              # rows per chunk / row-group
ROUND_C = 12582912.0    # 1.5 * 2**23: float32 round-half-even trick
EPS = 1e-8
S_SPEC = 2.0 ** -5      # speculative global scale (binade of max|x|/127+eps)
INV_SPEC = 2.0 ** 5


def _emit_scale_chain(nc, consts, gmax, gamma_sb, mask_t, expc_t, tag):
    """From a [P,1] absmax tile, compute s = exp2(floor(log2(m/127+eps)))
    via exponent masking, 1/s via exponent arithmetic, and c = s*gamma."""
    v_t = consts.tile([P, 1], F32, tag=f"v_{tag}")
    nc.vector.tensor_scalar(
        out=v_t,
        in0=gmax,
        scalar1=float(np.float32(1.0 / 127.0)),
        scalar2=float(np.float32(EPS)),
        op0=mybir.AluOpType.mult,
        op1=mybir.AluOpType.add,
    )
    s_t = consts.tile([P, 1], F32, tag=f"s_{tag}")
    nc.vector.tensor_tensor(
        out=s_t.bitcast(I32),
        in0=v_t.bitcast(I32),
        in1=mask_t,
        op=mybir.AluOpType.bitwise_and,
    )
    inv_t = consts.tile([P, 1], F32, tag=f"inv_{tag}")
    nc.vector.tensor_tensor(
        out=inv_t.bitcast(I32),
        in0=expc_t,
        in1=s_t.bitcast(I32),
        op=mybir.AluOpType.subtract,
    )
    c_t = consts.tile([P, 1], F32, tag=f"c_{tag}")
    nc.vector.tensor_mul(out=c_t, in0=s_t, in1=gamma_sb)
    return s_t, inv_t, c_t


def _emit_phase(nc, pools, nt, xc_tiles, yT, w_sb, bias_sb, inv, c_scale,
                rg_hook=None, store_engine=None, split_rg0=False):
    """Quantize x with 1/s (DVE mult+max, min+addC; ACT subC->bf16), then
    per row-group run the transposed matmul (W^T blocks stationary, xi rows
    streaming, PSUM partition dim = out_features), fuse scale+bias into the
    ACT PSUM->SBUF copy, store y^T tiles. `inv` is a float or [P,1] tile;
    `c_scale` a [P,1] tile. Returns last emitted instruction per engine.

    Stores go out on `store_engine` (default nc.gpsimd). The sync/HWDGE
    ring is preferred for the fast phase: SWDGE costs ~2us of Q7
    descriptor generation per dma, which backs up 64 stores and stalls
    the ACT copies on yo-pool recycling.

    rg_hook(rg, xc) is called one row-group LATE (after rg+1's quantize
    ops) so its absmax reduce never delays the next group's xi production.

    split_rg0 runs row-group 0's matmuls as two k-sweeps (k 0..3 then
    4..7, PSUM accumulation group held open across the sweeps) so the
    first 32 matmuls depend only on the first halves of w_sb and chunk 0.

    Quantize chain (exact): clip-before-round equals round-then-clip since
    the bounds are integers, so
      t1 = max(x*inv, -127); t1 = min(t1, 127) + C; xi = bf16(t1 - C)
    yields round-half-even(clip(x/s)) exactly (+-1.5*2^23 trick; mult by a
    power of two is exact; integer results |.|<=127 are bf16-exact)."""
    t1_pool, xi_pool, yo_pool, ps_pool = pools
    if store_engine is None:
        store_engine = nc.gpsimd
    last = {}

    def emit_quant(rg, pieces=2):
        """Quantize chunk rg in `pieces` equal k-ranges (finer pieces for
        chunk 0 let the first matmuls start before the whole chunk lands)."""
        xc = xc_tiles[rg]
        xi_slices = []
        kw = KC // pieces
        for h in range(pieces):
            t1 = t1_pool.tile([P, kw, RSUB], F32, tag="t1")
            nc.vector.tensor_scalar(
                out=t1,
                in0=xc[:, h * kw: (h + 1) * kw, :],
                scalar1=inv,
                scalar2=-127.0,
                op0=mybir.AluOpType.mult,
                op1=mybir.AluOpType.max,
            )
            last["DVE"] = nc.vector.tensor_scalar(
                out=t1,
                in0=t1,
                scalar1=127.0,
                scalar2=ROUND_C,
                op0=mybir.AluOpType.min,
                op1=mybir.AluOpType.add,
            )
            for kk in range(kw):
                xi = xi_pool.tile([P, RSUB], BF16, tag="xi")
                last["ACT"] = nc.scalar.activation(
                    out=xi,
                    in_=t1[:, kk, :],
                    func=mybir.ActivationFunctionType.Copy,
                    bias=-ROUND_C,
                    scale=1.0,
                )
                xi_slices.append(xi)
        return xi_slices

    def emit_mm_out(rg, xi_slices, sweeps=None):
        """sweeps: list of k-ranges; the PSUM accumulation group of each
        bank is held open across the sweeps so earlier sweeps can run
        before later k-slices (chunk-0 pipelining)."""
        if sweeps is None:
            sweeps = [(0, KC)]
        ps_tiles = []
        for j in range(JC):
            ps = ps_pool.tile([P, RSUB], F32, tag="ps")
            ps_tiles.append(ps)
        multi = len(sweeps) > 1
        for ka, kb in sweeps:
            for j in range(JC):
                for k in range(ka, kb):
                    last["PE"] = nc.tensor.matmul(
                        ps_tiles[j],
                        lhsT=w_sb[:, k, j * P: (j + 1) * P],
                        rhs=xi_slices[k],
                        start=(k == 0),
                        stop=(k == KC - 1),
                        skip_group_check=multi,
                    )
        for j in range(JC):
            yo = yo_pool.tile([P, RSUB], F32, tag="yo")
            last["ACT"] = nc.scalar.activation(
                out=yo,
                in_=ps_tiles[j],
                func=mybir.ActivationFunctionType.Identity,
                bias=bias_sb[:, j: j + 1],
                scale=c_scale,
            )
            last["ST"] = store_engine.dma_start(
                out=yT[j * P: (j + 1) * P, rg * RSUB: (rg + 1) * RSUB],
                in_=yo,
            )

    # Software pipeline with a one-group quantize lead: group rg's
    # quantize (DVE passes + ACT bf16 casts) is emitted BEFORE group
    # rg-1's matmul/copy/store section. The ACT engine's FIFO is strict;
    # without the lead, pass3(rg) sits behind copies(rg-1), which wait on
    # rg-1's matmuls - serializing xi production with PE progress.
    xi_prev = None
    for rg in range(nt):
        if split_rg0 and rg == 0:
            xi_now = emit_quant(0, pieces=4)
        else:
            xi_now = emit_quant(rg)
        if rg >= 1:
            # lagged absmax: after the NEXT group's quantize, off the
            # xi critical path
            if rg_hook is not None:
                rg_hook(rg - 1, xc_tiles[rg - 1])
            emit_mm_out(rg - 1, xi_prev,
                        sweeps=[(0, 2), (2, 4), (4, 8)]
                        if (split_rg0 and rg == 1) else None)
        xi_prev = xi_now
    if rg_hook is not None:
        rg_hook(nt - 1, xc_tiles[nt - 1])
    emit_mm_out(nt - 1, xi_prev)
    return last


def build_program(rows: int = 4096, num_cores: int = N_CORES,
                  safe: bool = False) -> bacc.Bacc:
    """safe=False: fast program - speculative scale hardcoded, binade-check
    verdict exported to DRAM (host re-runs the safe program on mispredict).
    safe=True: self-contained exact program with an on-device If/redo."""
    assert rows % RSUB == 0
    nc = bacc.Bacc(
        "TRN2",
        target_bir_lowering=False,
        debug=False,
        enable_asserts=False,
        num_devices=num_cores,
    )
    nt = rows // RSUB
    # x shard pre-tiled on host: xt[t, p, c, r] = x[t*RSUB + r, c*P + p],
    # so every chunk load is fully contiguous.
    xt = nc.dram_tensor("xt", (nt, P, KC, RSUB), F32, kind="ExternalInput").ap()
    wq = nc.dram_tensor("wq", (IN_F, OUT_F), BF16, kind="ExternalInput").ap()
    # bias pre-transposed on host to [P, JC]: bqt[p, j] = bq[j*128 + p]
    bqt = nc.dram_tensor("bqt", (P, JC), F32, kind="ExternalInput").ap()
    gq = nc.dram_tensor("gq", (1, 1), F32, kind="ExternalInput").ap()
    # transposed output y^T [out_features, rows]; host transposes back
    yT = nc.dram_tensor("yT", (OUT_F, rows), F32, kind="ExternalOutput").ap()
    # binade-check verdict (int32 bits; >= 0 means speculation missed)
    chk_out = nc.dram_tensor("chk", (P, 1), I32, kind="ExternalOutput").ap()
    # Collectives cannot target I/O tensors; bounce through internal DRAM.
    cc_in = nc.dram_tensor("cc_in", (P, 1), F32).ap()
    cc_out = nc.dram_tensor("cc_out", (P, 1), F32).ap()
    # dummy collective to pre-warm the ncfw/collectives firmware
    ccw_in = nc.dram_tensor("ccw_in", (P, 1), F32).ap()
    ccw_out = nc.dram_tensor("ccw_out", (P, 1), F32).ap()

    with tile.TileContext(nc, num_cores=num_cores) as tc, ExitStack() as ctx:
        consts = ctx.enter_context(tc.tile_pool(name="consts", bufs=1))

        mask_t = consts.tile([P, 1], I32)
        nc.vector.memset(mask_t, -8388608)  # 0xFF800000: sign+exponent mask
        expc_t = consts.tile([P, 1], I32)
        nc.vector.memset(expc_t, 0x7F000000)  # bits of (254<<23)

        # All big loads ride the sync ring (its HWDGE queue shards across
        # all 16 SDMA engines; the scalar ring's does not). Interleave the
        # weight halves with chunk 0's halves so the first matmul's inputs
        # land together as early as possible.
        gamma_sb = consts.tile([P, 1], F32)
        w_sb = consts.tile([P, KC, OUT_F], BF16)
        bias_sb = consts.tile([P, JC], F32)
        w_src = wq.rearrange("(c p) o -> p c o", p=P)
        c_spec = consts.tile([P, 1], F32)
        partials = consts.tile([P, nt], F32)
        gmax_g = consts.tile([P, 1], F32)
        warm_rhs = consts.tile([P, RSUB], BF16)
        nc.vector.memset(warm_rhs, 0.0)
        warm_f = consts.tile([P, 1], F32)
        nc.vector.memset(warm_f, 1.0)
        warm_o = consts.tile([P, 1], F32)
        rg_cc = [list(range(num_cores))]

        with (
            tc.tile_pool(name="xc", bufs=nt) as xc_pool,
            tc.tile_pool(name="t1", bufs=2) as t1_pool,
            tc.tile_pool(name="xi", bufs=3 * KC) as xi_pool,
            tc.tile_pool(name="yo", bufs=10) as yo_pool,
            tc.tile_pool(name="ps", bufs=8, space="PSUM") as ps_pool,
        ):
            pools = (t1_pool, xi_pool, yo_pool, ps_pool)

            # eager x loads: all chunks issued up front on the sync ring,
            # two half-chunk DMAs per chunk (finer landing granularity).
            # Issue order front-loads what the first matmul needs:
            # w half 0, chunk0 half 0, w half 1, chunk0 half 1, consts,
            # then chunks 1..nt-1.
            xc_tiles = []
            for _t in range(nt):
                xc = xc_pool.tile([P, KC, RSUB], F32, tag="xc")
                xc_tiles.append(xc)

            def load_x(t, ka, kb):
                return nc.sync.dma_start(
                    out=xc_tiles[t][:, ka:kb, :],
                    in_=xt[t, :, ka:kb, :],
                )

            def load_w(ka, kb):
                return nc.sync.dma_start(out=w_sb[:, ka:kb, :],
                                         in_=w_src[:, ka:kb, :])

            # quarter-granular interleave for what the first matmuls need
            nc.sync.dma_start(out=gamma_sb, in_=gq.to_broadcast((P, 1)))
            load_w(0, 2)
            load_x(0, 0, 2)
            load_w(2, 4)
            load_x(0, 2, 4)
            load_w(4, KC)
            load_x(0, 4, KC)
            nc.sync.dma_start(out=bias_sb, in_=bqt)
            for t in range(1, nt):
                load_x(t, 0, KC // 2)
                load_x(t, KC // 2, KC)

            # c = s_spec * gamma for the speculative fast path
            nc.vector.tensor_scalar(
                out=c_spec, in0=gamma_sb, scalar1=S_SPEC, scalar2=None,
                op0=mybir.AluOpType.mult,
            )

            # --- warmups, all dependency-free ---
            # PE: junk matmuls flip HAM to full clock before the first
            # real matmul
            warm_ps = ps_pool.tile([P, RSUB], F32, tag="ps")
            for _ in range(12):
                nc.tensor.matmul(
                    warm_ps, lhsT=warm_rhs[:, 0:P], rhs=warm_rhs,
                    start=True, stop=True,
                )
            # gpsimd: pre-load the Q7 partition-reduce library
            nc.gpsimd.partition_all_reduce(
                warm_o, warm_f, channels=P, reduce_op=bass_isa.ReduceOp.max,
            )
            # collectives firmware: dummy 512B AllReduce so the real one
            # later starts without the ncfw cold-wake latency
            nc.gpsimd.dma_start(out=ccw_in, in_=warm_f)
            nc.gpsimd.collective_compute(
                "AllReduce", mybir.AluOpType.max, replica_groups=rg_cc,
                ins=[ccw_in.opt()], outs=[ccw_out.opt()],
            )

            def rg_hook(rg, xc):
                # per-chunk absmax, off the critical path (reads the
                # resident x chunk); placed in the DVE stream inside rg's
                # segment so it never heads-of-line-blocks on a load
                nc.vector.tensor_reduce(
                    out=partials[:, rg: rg + 1],
                    in_=xc,
                    axis=mybir.AxisListType.XY,
                    op=mybir.AluOpType.max,
                    apply_absolute_value=True,
                )
                if rg == nt - 1:
                    # full local max -> 512B AllReduce(max) across the 8
                    # cores; completes well before the last matmul
                    lmax = consts.tile([P, 1], F32)
                    nc.vector.tensor_reduce(
                        out=lmax, in_=partials, axis=mybir.AxisListType.X,
                        op=mybir.AluOpType.max,
                    )
                    gmax_l = consts.tile([P, 1], F32)
                    nc.gpsimd.partition_all_reduce(
                        gmax_l, lmax, channels=P,
                        reduce_op=bass_isa.ReduceOp.max,
                    )
                    nc.gpsimd.dma_start(out=cc_in, in_=gmax_l)
                    nc.gpsimd.collective_compute(
                        "AllReduce", mybir.AluOpType.max, replica_groups=rg_cc,
                        ins=[cc_in.opt()], outs=[cc_out.opt()],
                    )
                    nc.gpsimd.dma_start(out=gmax_g, in_=cc_out)

            last = _emit_phase(
                nc, pools, nt, xc_tiles, yT, w_sb, bias_sb,
                INV_SPEC, c_spec, rg_hook=rg_hook,
                store_engine=nc.sync, split_rg0=True,
            )

            # --- speculation check: s_spec is the true scale iff
            # v = gmax/127+eps lies in [s_spec, 2*s_spec), i.e.
            # sign(v - s) != sign(v - 2s). XOR of the float bits makes the
            # pass condition a single sign test: chk < 0 <=> in-binade. ---
            v_g = consts.tile([P, 1], F32)
            nc.vector.tensor_scalar(
                out=v_g,
                in0=gmax_g,
                scalar1=float(np.float32(1.0 / 127.0)),
                scalar2=float(np.float32(EPS)),
                op0=mybir.AluOpType.mult,
                op1=mybir.AluOpType.add,
            )
            a_t = consts.tile([P, 1], F32)
            nc.vector.tensor_scalar(
                out=a_t, in0=v_g, scalar1=-S_SPEC, scalar2=None,
                op0=mybir.AluOpType.add,
            )
            b_t = consts.tile([P, 1], F32)
            nc.vector.tensor_scalar(
                out=b_t, in0=v_g, scalar1=-2.0 * S_SPEC, scalar2=None,
                op0=mybir.AluOpType.add,
            )
            chk = consts.tile([P, 1], I32)
            last["DVE"] = nc.vector.tensor_tensor(
                out=chk, in0=a_t.bitcast(I32), in1=b_t.bitcast(I32),
                op=mybir.AluOpType.bitwise_xor,
            )
            if not safe:
                # fast program: export the verdict; the host re-runs the
                # safe program in the (P < 1e-7) mispredict case. No
                # on-device branch means no skipped-region semaphore
                # reconciliation in the epilogue.
                nc.gpsimd.dma_start(out=chk_out, in_=chk)
            else:
                last["POOL"] = nc.gpsimd.dma_start(out=chk_out, in_=chk)
                regs = nc.alloc_registers(
                    "spec_chk",
                    bass.OrderedSet([
                        mybir.EngineType.SP,
                        mybir.EngineType.DVE,
                        mybir.EngineType.Activation,
                        mybir.EngineType.PE,
                        mybir.EngineType.Pool,
                    ]),
                )
                # Pin each engine's reg_load after its last speculative-
                # phase instruction: the load waits on the AllReduce, and
                # the Tile scheduler would otherwise be free to place it
                # mid-stream, stalling that engine's FIFO on the collective.
                eng_key = {
                    mybir.EngineType.PE: "PE",
                    mybir.EngineType.DVE: "DVE",
                    mybir.EngineType.Activation: "ACT",
                    mybir.EngineType.SP: "ST",
                    mybir.EngineType.Pool: "POOL",
                }
                for reg in regs:
                    ld = nc.reg_load(reg, chk[0:1, 0:1])
                    prev = last.get(eng_key[reg.engine])
                    if prev is not None:
                        tile.add_dep_helper(
                            ld.ins, prev.ins, sync=False,
                            reason="speculation check after spec phase",
                        )
                # negative iff inside the speculated binade (fast path)
                with tc.If(nc.snap(regs) >= 0):
                    # mismatch: redo with the exact global scale
                    # (x chunks are still resident in SBUF - no reloads)
                    _, inv_g, c_g = _emit_scale_chain(
                        nc, consts, gmax_g, gamma_sb, mask_t, expc_t, "g")
                    _emit_phase(nc, pools, nt, xc_tiles, yT, w_sb, bias_sb,
                                inv_g, c_g, store_engine=nc.sync)

    nc.compile()
    return nc


def quantize_params(weight: np.ndarray, bias: np.ndarray):
    """Ternary-quantize weight/bias exactly as the reference (f64 math whose
    f32 rounding matches jax-f32; verified margins are orders of magnitude
    above f32 accumulation differences)."""
    w64 = weight.astype(np.float64)
    g_w = np.float32(np.abs(w64).mean())
    wi = np.clip(np.round(w64 / (np.float64(g_w) + EPS)), -1.0, 1.0)
    b64 = bias.astype(np.float64)
    g_b = np.float32(np.abs(b64).mean())
    bi = np.clip(np.round(b64 / (np.float64(g_b) + EPS)), -1.0, 1.0)
    bq = (bi * np.float64(g_b)).astype(np.float32)  # exact: {-g_b, 0, g_b}
    return wi, g_w, bq


_PROGRAM_CACHE: dict[tuple[int, bool], bacc.Bacc] = {}


def _get_program(rows: int, safe: bool = False) -> bacc.Bacc:
    key = (rows, safe)
    if key not in _PROGRAM_CACHE:
        _PROGRAM_CACHE[key] = build_program(rows, safe=safe)
    return _PROGRAM_CACHE[key]


def tile_x_shard(x2d: np.ndarray) -> np.ndarray:
    """[rows, IN_F] -> [nt, P, KC, RSUB] with xt[t,p,c,r] = x[t*RSUB+r, c*P+p]."""
    rows = x2d.shape[0]
    return np.ascontiguousarray(
        x2d.reshape(rows // RSUB, RSUB, KC, P).transpose(0, 3, 2, 1)
    )


def prepare_in_maps(x: np.ndarray, weight: np.ndarray, bias: np.ndarray):
    x = np.asarray(x, dtype=np.float32)
    weight = np.asarray(weight, dtype=np.float32)
    bias = np.asarray(bias, dtype=np.float32)
    batch, rows, in_f = x.shape
    assert batch == N_CORES and in_f == IN_F and weight.shape == (OUT_F, IN_F)

    wi, g_w, bq = quantize_params(weight, bias)
    wq_t = np.ascontiguousarray(wi.T).astype(ml_dtypes.bfloat16)  # [in, out]
    bqt = np.ascontiguousarray(bq.reshape(JC, P).T)               # [P, JC]
    gq = np.array([[g_w]], dtype=np.float32)

    in_maps = []
    for c in range(N_CORES):
        in_maps.append(
            {
                "xt": tile_x_shard(x[c]),
                "wq": wq_t,
                "bqt": bqt,
                "gq": gq,
            }
        )
    return in_maps, rows


def kernel(x: np.ndarray, weight: np.ndarray, bias: np.ndarray) -> np.ndarray:
    in_maps, rows = prepare_in_maps(x, weight, bias)
    nc = _get_program(rows)
    res = bass_utils.run_bass_kernel_spmd(nc, in_maps, core_ids=list(range(N_CORES)))
    # device-computed binade check: int32 bits of the XOR sign test are
    # negative iff the speculated scale is the true global scale
    if any(int(res.results[c]["chk"][0, 0]) >= 0 for c in range(N_CORES)):
        # speculation missed (P < 1e-7 for randn inputs): run the
        # self-contained exact program with the on-device redo branch
        nc_safe = _get_program(rows, safe=True)
        res = bass_utils.run_bass_kernel_spmd(
            nc_safe, in_maps, core_ids=list(range(N_CORES)))
    return np.stack(
        [np.ascontiguousarray(res.results[c]["yT"].T) for c in range(N_CORES)],
        axis=0,
    )
